# revision 11
# baseline (speedup 1.0000x reference)
"""GAT 2-layer + BN + classifier on 8 TRN2 NeuronCores (Bass/Tile).

Strategy: dst-block sharding with per-core table rotation so the SPMD
instruction stream is identical across cores. 5 launches:
  L1   node: h1_aug = x @ [W1 | W1@as1 | W1@ad1], table rows [h|as|ad|1|0]
  E(1) edge: gather h1_aug[src] per edge subtile, segment softmax via
       selection-matrix matmul in PSUM, partial BN stats
  L3   node: BN1 apply + relu + @W2_aug -> h2_aug table shard
  E(2) edge: same NEFF as E(1) on h2_aug
  L5   node: BN2 apply + relu + @Wc + bc -> logits shard
Host work is index-only: edge sort/shard, table assembly/rotation.
"""
import sys
sys.path.insert(0, '/opt/trn_rl_repo')
sys.path.insert(0, '/root/.axon_site')
import numpy as np

import concourse.bass as bass
import concourse.bacc as bacc
import concourse.tile as tile
from concourse import mybir
from concourse.masks import make_identity

F32 = mybir.dt.float32
I32 = mybir.dt.int32

N = 100000
NCORE = 8
BLK = 128
NPAD = 100352            # 784 blocks of 128
PC = NPAD // NCORE       # 12544 nodes/core = 98 blocks
NBLK = PC // BLK         # 98
TCOLS = 192              # table row: [h(128)|as|ad|one|pad...] (768B, 256B-divisible for dma_gather)
SUBT = 25088             # int16 subtable size (= 196 blocks)
NSUBT = 4
HID = 128
NCLS = 40
NEG = 0.2
EPS = 1e-5

_EXEC_NS = []            # per-launch exec times when profiling enabled
PROFILE = False


def _run(nc, in_maps, label):
    from trnprof import profiled_run
    res, ns, _ = profiled_run(nc, in_maps, n_cores=NCORE, profile=PROFILE)
    if PROFILE:
        _EXEC_NS.append((label, ns))
    return res


# ---------------------------------------------------------------- L1 node
def build_l1():
    nc = bacc.Bacc("TRN2", target_bir_lowering=False, debug=False, num_devices=NCORE)
    xT = nc.dram_tensor("xT", [128, NPAD], F32, kind="ExternalInput")
    W1 = nc.dram_tensor("W1", [128, HID], F32, kind="ExternalInput")
    avec = nc.dram_tensor("avec", [128, 2], F32, kind="ExternalInput")  # [as|ad] cols
    out = nc.dram_tensor("out", [PC, TCOLS], F32, kind="ExternalOutput")

    with tile.TileContext(nc) as tc:
        with (
            tc.tile_pool(name="c", bufs=1) as cp,
            tc.tile_pool(name="x", bufs=3) as xp,
            tc.tile_pool(name="o", bufs=3) as op,
            tc.tile_pool(name="ps", bufs=2, space="PSUM") as pp,
            tc.tile_pool(name="ps1", bufs=1, space="PSUM") as pp1,
        ):
            ident = cp.tile([128, 128], F32)
            make_identity(nc, ident[:])
            w_sb = cp.tile([128, HID], F32)
            nc.sync.dma_start(w_sb[:], W1[:])
            av_sb = cp.tile([128, 2], F32)
            nc.sync.dma_start(av_sb[:], avec[:])
            # W1T for v = W1 @ a
            wT_ps = pp1.tile([128, 128], F32, tag="tmp")
            nc.tensor.transpose(out=wT_ps[:], in_=w_sb[:], identity=ident[:])
            wT_sb = cp.tile([128, 128], F32)
            nc.vector.tensor_copy(out=wT_sb[:], in_=wT_ps[:])
            v_ps = pp1.tile([128, 2], F32, tag="tmp")
            nc.tensor.matmul(out=v_ps[:], lhsT=wT_sb[:], rhs=av_sb[:], start=True, stop=True)
            waug = cp.tile([128, HID + 2], F32)
            nc.vector.tensor_copy(out=waug[:, 0:HID], in_=w_sb[:])
            nc.vector.tensor_copy(out=waug[:, HID:HID + 2], in_=v_ps[:])
            for t in range(NBLK):
                xt = xp.tile([128, 128], F32, tag="x")
                nc.sync.dma_start(xt[:], xT[:, t * 128:(t + 1) * 128])
                h_ps = pp.tile([128, HID + 2], F32, tag="h")
                nc.tensor.matmul(out=h_ps[:], lhsT=xt[:], rhs=waug[:], start=True, stop=True)
                ot = op.tile([128, TCOLS], F32, tag="o")
                nc.vector.tensor_copy(out=ot[:, 0:HID + 2], in_=h_ps[:])
                nc.vector.memset(ot[:, HID + 2:HID + 3], 1.0)
                nc.vector.memset(ot[:, HID + 3:TCOLS], 0.0)
                nc.sync.dma_start(out[t * 128:(t + 1) * 128, :], ot[:])
    nc.compile()
    return nc


# ---------------------------------------------------------------- edge kernel
def build_edge(seg_counts):
    """seg_counts[t][k]: subtile count for block-slot t, subtable k (shared)."""
    from concourse.library_config import mlp
    seg_counts = [list(map(int, r)) for r in seg_counts]
    t_tot = [sum(r) for r in seg_counts]
    nsub = int(sum(t_tot))
    TMAXB = max(t_tot)
    GRP = 4
    groups = [list(range(g0, min(g0 + GRP, NBLK))) for g0 in range(0, NBLK, GRP)]
    gk_sub = [[sum(seg_counts[t][k] for t in grp) for k in range(NSUBT)]
              for grp in groups]
    MAXSUB = [max(gk_sub[gi][k] for gi in range(len(groups))) for k in range(NSUBT)]
    NW = int(sum(sum(r) for r in gk_sub) * 8)  # idx16 cols: 8 per subtile

    nc = bacc.Bacc("TRN2", target_bir_lowering=False, debug=False,
                   num_devices=NCORE, num_swdge_queues=4)
    table = nc.dram_tensor("table", [NPAD, TCOLS], F32, kind="ExternalInput")
    idx16 = nc.dram_tensor("idx16", [128, NW], mybir.dt.int16, kind="ExternalInput")
    dst_loc = nc.dram_tensor("dst_loc", [128, nsub], F32, kind="ExternalInput")
    agg = nc.dram_tensor("agg", [PC, HID], F32, kind="ExternalOutput")
    stats = nc.dram_tensor("stats", [1, 256], F32, kind="ExternalOutput")

    with tile.TileContext(nc) as tc:
        with (
            tc.tile_pool(name="c", bufs=1) as cp,
            tc.tile_pool(name="g", bufs=2) as gp,
            tc.tile_pool(name="s0", bufs=26) as s0p,
            tc.tile_pool(name="sw", bufs=3) as swp,
            tc.tile_pool(name="w", bufs=2) as wp,
            tc.tile_pool(name="ob", bufs=3) as obp,
            tc.tile_pool(name="own", bufs=2) as ownp,
            tc.tile_pool(name="pblk", bufs=2, space="PSUM") as pblk,
            tc.tile_pool(name="pal", bufs=2, space="PSUM") as pal,
            tc.tile_pool(name="ptr", bufs=2, space="PSUM") as ptr,
            tc.tile_pool(name="pst", bufs=1, space="PSUM") as pst,
        ):
            nc.gpsimd.load_library(mlp)
            ident = cp.tile([128, 128], F32)
            make_identity(nc, ident[:])
            iota_i = cp.tile([128, 128], I32)
            nc.gpsimd.iota(iota_i[:], pattern=[[1, 128]], base=0, channel_multiplier=0)
            iota_f = cp.tile([128, 128], F32)
            nc.vector.tensor_copy(out=iota_f[:], in_=iota_i[:])
            ones_col = cp.tile([128, 1], F32)
            nc.vector.memset(ones_col[:], 1.0)
            idx_sb = cp.tile([128, NW], mybir.dt.int16)
            nc.sync.dma_start(idx_sb[:], idx16[:])
            dl_sb = cp.tile([128, nsub], F32)
            nc.sync.dma_start(dl_sb[:], dst_loc[:])

            ps_sum = pst.tile([1, 128], F32, tag="sum")
            ps_sq = pst.tile([1, 128], F32, tag="sq")

            qoff = [0]
            for t in range(NBLK):
                qoff.append(qoff[-1] + t_tot[t])

            c0 = 0
            call_col = {}   # (gi, k) -> idx16 col offset
            for gi in range(len(groups)):
                for k in range(NSUBT):
                    call_col[(gi, k)] = c0
                    c0 += gk_sub[gi][k] * 8

            for gi, grp in enumerate(groups):
                gts = []
                for k in range(NSUBT):
                    gt = gp.tile([128, MAXSUB[k], TCOLS], F32, tag=f"g{k}",
                                 name=f"gt{gi}_{k}")
                    sub = gk_sub[gi][k]
                    if sub > 0:
                        cc = call_col[(gi, k)]
                        nc.gpsimd.dma_gather(
                            gt[:, 0:sub, :], table[SUBT * k:SUBT * (k + 1), :],
                            idx_sb[:, cc:cc + sub * 8],
                            sub * 128, sub * 128, TCOLS,
                            single_packet=False, queue_num=k)
                    gts.append(gt)
                for t in grp:
                    T = t_tot[t]
                    ownt = ownp.tile([128, 4], F32, tag="own", name=f"own{t}")
                    nc.sync.dma_start(ownt[:], table[t * 128:(t + 1) * 128, HID:HID + 4])
                    ps_a = pal.tile([128, TMAXB], F32, tag="al", name=f"al{t}")
                    # per-subtile refs: (k, call subtile index)
                    refs = []
                    for k in range(NSUBT):
                        soff = sum(seg_counts[tt][k] for tt in grp if tt < t)
                        for s in range(seg_counts[t][k]):
                            refs.append((k, soff + s))
                    s0_list = []
                    for j, (k, cs) in enumerate(refs):
                        q = qoff[t] + j
                        s0 = s0p.tile([128, 128], F32, tag="s0", name=f"s0_{t}_{j}")
                        nc.vector.tensor_scalar(
                            out=s0[:], in0=iota_f[:], scalar1=dl_sb[:, q:q + 1],
                            scalar2=None, op0=mybir.AluOpType.is_equal)
                        s0_list.append(s0)
                        s0t_ps = ptr.tile([128, 128], F32, tag="tr", name=f"tr{t}_{j}")
                        nc.tensor.transpose(out=s0t_ps[:], in_=s0[:], identity=ident[:])
                        s0t = swp.tile([128, 128], F32, tag="s0t", name=f"s0t{t}_{j}")
                        nc.vector.tensor_copy(out=s0t[:], in_=s0t_ps[:])
                        nc.tensor.matmul(out=ps_a[:, j:j + 1], lhsT=s0t[:],
                                         rhs=ownt[:, 1:2], start=True, stop=True)
                    w_blk = wp.tile([128, TMAXB], F32, tag="w", name=f"w{t}")
                    ps_b = pblk.tile([128, HID + 3], F32, tag="blk", name=f"blk{t}")
                    for j, (k, cs) in enumerate(refs):
                        nc.vector.tensor_tensor(
                            out=w_blk[:, j:j + 1], in0=gts[k][:, cs, HID:HID + 1],
                            in1=ps_a[:, j:j + 1], op=mybir.AluOpType.add)
                    wb2 = wp.tile([128, TMAXB], F32, tag="w2", name=f"w2_{t}")
                    nc.vector.tensor_scalar(out=wb2[:, 0:T], in0=w_blk[:, 0:T],
                                            scalar1=NEG, scalar2=None,
                                            op0=mybir.AluOpType.mult)
                    nc.vector.tensor_tensor(out=w_blk[:, 0:T], in0=w_blk[:, 0:T],
                                            in1=wb2[:, 0:T], op=mybir.AluOpType.max)
                    nc.scalar.activation(out=w_blk[:, 0:T], in_=w_blk[:, 0:T],
                                         func=mybir.ActivationFunctionType.Exp)
                    for j, (k, cs) in enumerate(refs):
                        sw = swp.tile([128, 128], F32, tag="sw", name=f"sw{t}_{j}")
                        nc.vector.tensor_scalar(
                            out=sw[:], in0=s0_list[j][:], scalar1=w_blk[:, j:j + 1],
                            scalar2=None, op0=mybir.AluOpType.mult)
                        nc.tensor.matmul(out=ps_b[:], lhsT=sw[:],
                                         rhs=gts[k][:, cs, 0:HID + 3],
                                         start=(j == 0), stop=(j == T - 1))
                    den = wp.tile([128, 1], F32, tag="den", name=f"den{t}")
                    nc.vector.tensor_scalar(out=den[:], in0=ps_b[:, HID + 2:HID + 3],
                                            scalar1=0.0, scalar2=None,
                                            op0=mybir.AluOpType.is_equal)
                    nc.vector.tensor_tensor(out=den[:], in0=den[:],
                                            in1=ps_b[:, HID + 2:HID + 3],
                                            op=mybir.AluOpType.add)
                    rec = wp.tile([128, 1], F32, tag="rec", name=f"rec{t}")
                    nc.vector.reciprocal(out=rec[:], in_=den[:])
                    ob = obp.tile([128, HID], F32, tag="ob", name=f"ob{t}")
                    nc.vector.tensor_scalar(out=ob[:], in0=ps_b[:, 0:HID], scalar1=rec[:],
                                            scalar2=None, op0=mybir.AluOpType.mult)
                    nc.sync.dma_start(agg[t * 128:(t + 1) * 128, :], ob[:])
                    sq = obp.tile([128, HID], F32, tag="sq", name=f"sq{t}")
                    nc.scalar.activation(out=sq[:], in_=ob[:],
                                         func=mybir.ActivationFunctionType.Square)
                    nc.tensor.matmul(out=ps_sum[:], lhsT=ones_col[:], rhs=ob[:],
                                     start=(t == 0), stop=(t == NBLK - 1))
                    nc.tensor.matmul(out=ps_sq[:], lhsT=ones_col[:], rhs=sq[:],
                                     start=(t == 0), stop=(t == NBLK - 1))
            st_sb = cp.tile([1, 256], F32)
            nc.vector.tensor_copy(out=st_sb[:, 0:128], in_=ps_sum[:])
            nc.vector.tensor_copy(out=st_sb[:, 128:256], in_=ps_sq[:])
            nc.sync.dma_start(stats[:], st_sb[:])
    nc.compile()
    return nc, groups, gk_sub, call_col


# ---------------------------------------------------------------- node tail
def build_node2(classifier):
    """BN apply + relu (+ next-layer table build, or classifier)."""
    nc = bacc.Bacc("TRN2", target_bir_lowering=False, debug=False, num_devices=NCORE)
    agg = nc.dram_tensor("agg", [PC, HID], F32, kind="ExternalInput")
    parts = nc.dram_tensor("parts", [8, 256], F32, kind="ExternalInput")
    gb = nc.dram_tensor("gb", [1, 256], F32, kind="ExternalInput")  # [gamma|beta]
    if classifier:
        Wn = nc.dram_tensor("Wn", [128, NCLS], F32, kind="ExternalInput")
        bc = nc.dram_tensor("bc", [1, NCLS], F32, kind="ExternalInput")
        out = nc.dram_tensor("out", [PC, NCLS], F32, kind="ExternalOutput")
    else:
        Wn = nc.dram_tensor("Wn", [128, HID], F32, kind="ExternalInput")
        avec = nc.dram_tensor("avec", [128, 2], F32, kind="ExternalInput")
        out = nc.dram_tensor("out", [PC, TCOLS], F32, kind="ExternalOutput")

    with tile.TileContext(nc) as tc:
        with (
            tc.tile_pool(name="c", bufs=1) as cp,
            tc.tile_pool(name="x", bufs=3) as xp,
            tc.tile_pool(name="o", bufs=3) as op,
            tc.tile_pool(name="ps", bufs=2, space="PSUM") as pp,
            tc.tile_pool(name="ps1", bufs=2, space="PSUM") as pp1,
        ):
            ident = cp.tile([128, 128], F32)
            make_identity(nc, ident[:])
            parts_sb = cp.tile([8, 256], F32)
            nc.sync.dma_start(parts_sb[:], parts[:])
            ones8 = cp.tile([8, 1], F32)
            nc.vector.memset(ones8[:], 1.0)
            st_ps = pp1.tile([1, 256], F32, tag="tmp")
            nc.tensor.matmul(out=st_ps[:], lhsT=ones8[:], rhs=parts_sb[:], start=True, stop=True)
            stat = cp.tile([1, 256], F32)
            nc.vector.tensor_scalar(out=stat[:], in0=st_ps[:], scalar1=1.0 / N,
                                    scalar2=None, op0=mybir.AluOpType.mult)
            mean = stat[:, 0:128]
            msq = stat[:, 128:256]
            var = cp.tile([1, 128], F32)
            nc.vector.tensor_tensor(out=var[:], in0=mean, in1=mean, op=mybir.AluOpType.mult)
            nc.vector.tensor_tensor(out=var[:], in0=msq, in1=var[:], op=mybir.AluOpType.subtract)
            nc.vector.tensor_scalar(out=var[:], in0=var[:], scalar1=EPS,
                                    scalar2=None, op0=mybir.AluOpType.add)
            std = cp.tile([1, 128], F32)
            nc.scalar.activation(out=std[:], in_=var[:],
                                 func=mybir.ActivationFunctionType.Sqrt)
            istd = cp.tile([1, 128], F32)
            nc.vector.reciprocal(out=istd[:], in_=std[:])
            gb_sb = cp.tile([1, 256], F32)
            nc.sync.dma_start(gb_sb[:], gb[:])
            gam = cp.tile([1, 128], F32)
            nc.vector.tensor_tensor(out=gam[:], in0=gb_sb[:, 0:128], in1=istd[:],
                                    op=mybir.AluOpType.mult)
            bet = cp.tile([1, 128], F32)
            nc.vector.tensor_tensor(out=bet[:], in0=mean, in1=gam[:], op=mybir.AluOpType.mult)
            nc.vector.tensor_tensor(out=bet[:], in0=gb_sb[:, 128:256], in1=bet[:],
                                    op=mybir.AluOpType.subtract)
            # broadcast gamma', beta' to [128, 128] via K=1 matmul
            one1 = cp.tile([1, 128], F32)
            nc.vector.memset(one1[:], 1.0)
            gbc_ps = pp1.tile([128, 128], F32, tag="tmp")
            nc.tensor.matmul(out=gbc_ps[:], lhsT=one1[:], rhs=gam[:], start=True, stop=True)
            gbc = cp.tile([128, 128], F32)
            nc.vector.tensor_copy(out=gbc[:], in_=gbc_ps[:])
            bbc_ps = pp1.tile([128, 128], F32, tag="tmp")
            nc.tensor.matmul(out=bbc_ps[:], lhsT=one1[:], rhs=bet[:], start=True, stop=True)
            bbc = cp.tile([128, 128], F32)
            nc.vector.tensor_copy(out=bbc[:], in_=bbc_ps[:])

            if classifier:
                wn_sb = cp.tile([128, NCLS], F32)
                nc.sync.dma_start(wn_sb[:], Wn[:])
                bc_sb = cp.tile([1, NCLS], F32)
                nc.sync.dma_start(bc_sb[:], bc[:])
                bcb_ps = pp1.tile([128, NCLS], F32, tag="tmp")
                nc.tensor.matmul(out=bcb_ps[:], lhsT=one1[:], rhs=bc_sb[:], start=True, stop=True)
                bcb = cp.tile([128, NCLS], F32)
                nc.vector.tensor_copy(out=bcb[:], in_=bcb_ps[:])
                rhs_w = wn_sb
                ncols = NCLS
            else:
                wn_sb = cp.tile([128, HID], F32)
                nc.sync.dma_start(wn_sb[:], Wn[:])
                av_sb = cp.tile([128, 2], F32)
                nc.sync.dma_start(av_sb[:], avec[:])
                wT_ps = pp1.tile([128, 128], F32, tag="tmp")
                nc.tensor.transpose(out=wT_ps[:], in_=wn_sb[:], identity=ident[:])
                wT_sb = cp.tile([128, 128], F32)
                nc.vector.tensor_copy(out=wT_sb[:], in_=wT_ps[:])
                v_ps = pp1.tile([128, 2], F32, tag="tmp")
                nc.tensor.matmul(out=v_ps[:], lhsT=wT_sb[:], rhs=av_sb[:], start=True, stop=True)
                waug = cp.tile([128, HID + 2], F32)
                nc.vector.tensor_copy(out=waug[:, 0:HID], in_=wn_sb[:])
                nc.vector.tensor_copy(out=waug[:, HID:HID + 2], in_=v_ps[:])
                rhs_w = waug
                ncols = HID + 2

            for t in range(NBLK):
                at = xp.tile([128, HID], F32, tag="a")
                nc.sync.dma_start(at[:], agg[t * 128:(t + 1) * 128, :])
                x2 = xp.tile([128, HID], F32, tag="x2")
                nc.vector.tensor_tensor(out=x2[:], in0=at[:], in1=gbc[:], op=mybir.AluOpType.mult)
                nc.vector.tensor_tensor(out=x2[:], in0=x2[:], in1=bbc[:], op=mybir.AluOpType.add)
                nc.scalar.activation(out=x2[:], in_=x2[:],
                                     func=mybir.ActivationFunctionType.Relu)
                xT_ps = pp.tile([128, 128], F32, tag="xt")
                nc.tensor.transpose(out=xT_ps[:], in_=x2[:], identity=ident[:])
                xT_sb = xp.tile([128, 128], F32, tag="xts")
                nc.vector.tensor_copy(out=xT_sb[:], in_=xT_ps[:])
                h_ps = pp.tile([128, ncols], F32, tag="h")
                nc.tensor.matmul(out=h_ps[:], lhsT=xT_sb[:], rhs=rhs_w[:], start=True, stop=True)
                if classifier:
                    ot = op.tile([128, NCLS], F32, tag="o")
                    nc.vector.tensor_tensor(out=ot[:], in0=h_ps[:], in1=bcb[:],
                                            op=mybir.AluOpType.add)
                    nc.sync.dma_start(out[t * 128:(t + 1) * 128, :], ot[:])
                else:
                    ot = op.tile([128, TCOLS], F32, tag="o")
                    nc.vector.tensor_copy(out=ot[:, 0:HID + 2], in_=h_ps[:])
                    nc.vector.memset(ot[:, HID + 2:HID + 3], 1.0)
                    nc.vector.memset(ot[:, HID + 3:TCOLS], 0.0)
                    nc.sync.dma_start(out[t * 128:(t + 1) * 128, :], ot[:])
    nc.compile()
    return nc


# ---------------------------------------------------------------- host glue
def _edge_arrays(src, dst):
    """seg_counts [NBLK,NSUBT] + per-core idx16 (wrapped int16) + dst_local."""
    order = np.argsort(dst, kind="stable")
    srcs = src[order]
    dsts = dst[order]
    counts = np.bincount(dsts // BLK, minlength=NPAD // BLK)
    starts = np.concatenate([[0], np.cumsum(counts)])
    GRP = 4
    lists = [[None] * NSUBT for _ in range(NCORE * NBLK)]
    cnts = np.zeros((NCORE, NBLK, NSUBT), np.int64)
    for c in range(NCORE):
        for t in range(NBLK):
            b = c * NBLK + t
            sl = slice(starts[b], starts[b + 1])
            s_ = srcs[sl]
            d_ = dsts[sl]
            rolled = (s_ - PC * c) % NPAD
            k_ = rolled // SUBT
            for k in range(NSUBT):
                m = k_ == k
                lists[b][k] = ((rolled[m] - SUBT * k).astype(np.int64),
                               (d_[m] - b * BLK).astype(np.float32))
                cnts[c, t, k] = int(m.sum())
    seg_counts = np.ceil(cnts.max(axis=0) / BLK).astype(np.int64)  # [NBLK, NSUBT]
    groups = [list(range(g0, min(g0 + GRP, NBLK))) for g0 in range(0, NBLK, GRP)]
    gk_sub = [[int(sum(seg_counts[t][k] for t in grp)) for k in range(NSUBT)]
              for grp in groups]
    NW = int(sum(sum(r) for r in gk_sub) * 8)
    t_tot = seg_counts.sum(axis=1)
    nsub = int(t_tot.sum())
    qoff = np.concatenate([[0], np.cumsum(t_tot)]).astype(np.int64)
    call_col = {}
    c0 = 0
    for gi in range(len(groups)):
        for k in range(NSUBT):
            call_col[(gi, k)] = c0
            c0 += gk_sub[gi][k] * 8
    idx_arrs, dst_arrs = [], []
    for c in range(NCORE):
        iw = np.zeros((16, NW), np.int16)
        dl = np.full((128, nsub), 200.0, np.float32)
        for gi, grp in enumerate(groups):
            for k in range(NSUBT):
                sub = gk_sub[gi][k]
                if sub == 0:
                    continue
                flat = np.zeros(sub * 128, np.int64)
                pos = 0
                for t in grp:
                    loc, _ = lists[c * NBLK + t][k]
                    flat[pos:pos + len(loc)] = loc
                    pos += int(seg_counts[t][k]) * 128
                cc = call_col[(gi, k)]
                iw[:, cc:cc + sub * 8] = flat.reshape(sub * 8, 16).T.astype(np.int16)
        for t in range(NBLK):
            joff = 0
            for k in range(NSUBT):
                _, dloc = lists[c * NBLK + t][k]
                kk = np.arange(len(dloc))
                dl[kk % 128, qoff[t] + joff + kk // 128] = dloc
                joff += int(seg_counts[t][k])
        idx_arrs.append(np.tile(iw, (8, 1)))
        dst_arrs.append(dl)
    return seg_counts, idx_arrs, dst_arrs


_CACHE = {}


def kernel(x, edge_index, W1, as1, ad1, b1, g1, beta1,
           W2, as2, ad2, b2, g2, beta2, Wc, bc):
    x = np.asarray(x, np.float32)
    ei = np.asarray(edge_index)
    src = np.concatenate([ei[0], np.arange(N, dtype=ei.dtype)]).astype(np.int64)
    dst = np.concatenate([ei[1], np.arange(N, dtype=ei.dtype)]).astype(np.int64)

    seg_counts, idx_arrs, dst_arrs = _edge_arrays(src, dst)

    key = seg_counts.tobytes()
    if key not in _CACHE:
        _CACHE[key] = (build_l1(), build_edge(seg_counts)[0],
                       build_node2(False), build_node2(True))
    nc1, nce, nc3, nc5 = _CACHE[key]

    # ---- L1
    xT = np.zeros((128, NPAD), np.float32)
    xT[:, :N] = np.asarray(x, np.float32).T
    av = np.stack([np.asarray(as1, np.float32), np.asarray(ad1, np.float32)], axis=1)
    in1 = [{"xT": np.roll(xT, -PC * c, axis=1).copy(),
            "W1": np.asarray(W1, np.float32), "avec": av} for c in range(NCORE)]
    r1 = _run(nc1, in1, "L1")
    h1 = np.concatenate([r1[c]["out"] for c in range(NCORE)], axis=0)  # [NPAD, 132]

    # ---- E1
    ine = [{"table": np.roll(h1, -PC * c, axis=0).copy(),
            "idx16": idx_arrs[c], "dst_loc": dst_arrs[c]} for c in range(NCORE)]
    re1 = _run(nce, ine, "E1")
    agg1 = [re1[c]["agg"] for c in range(NCORE)]
    parts1 = np.stack([re1[c]["stats"][0] for c in range(NCORE)], axis=0)  # [8, 256]

    # ---- L3
    gb1 = np.concatenate([np.asarray(g1, np.float32),
                          np.asarray(beta1, np.float32)])[None, :]
    av2 = np.stack([np.asarray(as2, np.float32), np.asarray(ad2, np.float32)], axis=1)
    in3 = [{"agg": agg1[c], "parts": parts1, "gb": gb1,
            "Wn": np.asarray(W2, np.float32), "avec": av2} for c in range(NCORE)]
    r3 = _run(nc3, in3, "L3")
    h2 = np.concatenate([r3[c]["out"] for c in range(NCORE)], axis=0)

    # ---- E2
    ine2 = [{"table": np.roll(h2, -PC * c, axis=0).copy(),
             "idx16": idx_arrs[c], "dst_loc": dst_arrs[c]} for c in range(NCORE)]
    re2 = _run(nce, ine2, "E2")
    agg2 = [re2[c]["agg"] for c in range(NCORE)]
    parts2 = np.stack([re2[c]["stats"][0] for c in range(NCORE)], axis=0)

    # ---- L5
    gb2 = np.concatenate([np.asarray(g2, np.float32),
                          np.asarray(beta2, np.float32)])[None, :]
    in5 = [{"agg": agg2[c], "parts": parts2, "gb": gb2,
            "Wn": np.asarray(Wc, np.float32),
            "bc": np.asarray(bc, np.float32)[None, :]} for c in range(NCORE)]
    r5 = _run(nc5, in5, "L5")
    logits = np.concatenate([r5[c]["out"] for c in range(NCORE)], axis=0)
    return logits[:N]


# revision 12
# speedup vs baseline: 1.2429x; 1.2429x over previous
"""GAT 2-layer + BN + classifier on 8 TRN2 NeuronCores (Bass/Tile).

Strategy: dst-block sharding with per-core table rotation so the SPMD
instruction stream is identical across cores. 5 launches:
  L1   node: h1_aug = x @ [W1 | W1@as1 | W1@ad1], table rows [h|as|ad|1|0]
  E(1) edge: gather h1_aug[src] per edge subtile, segment softmax via
       selection-matrix matmul in PSUM, partial BN stats
  L3   node: BN1 apply + relu + @W2_aug -> h2_aug table shard
  E(2) edge: same NEFF as E(1) on h2_aug
  L5   node: BN2 apply + relu + @Wc + bc -> logits shard
Host work is index-only: edge sort/shard, table assembly/rotation.
"""
import sys
sys.path.insert(0, '/opt/trn_rl_repo')
sys.path.insert(0, '/root/.axon_site')
import numpy as np

import concourse.bass as bass
import concourse.bacc as bacc
import concourse.tile as tile
from concourse import mybir
from concourse.masks import make_identity

F32 = mybir.dt.float32
I32 = mybir.dt.int32

N = 100000
NCORE = 8
BLK = 128
NPAD = 100352            # 784 blocks of 128
PC = NPAD // NCORE       # 12544 nodes/core = 98 blocks
NBLK = PC // BLK         # 98
TCOLS = 132              # table row: [h(128) | as | ad | one | pad]
HID = 128
NCLS = 40
NEG = 0.2
EPS = 1e-5

_EXEC_NS = []            # per-launch exec times when profiling enabled
PROFILE = False


RUN_HOOK = None          # test harness may set this to a profiling runner


def _run(nc, in_maps, label):
    if RUN_HOOK is not None:
        return RUN_HOOK(nc, in_maps, label)
    from concourse import bass2jax
    return bass2jax.run_bass_via_pjrt(nc, in_maps, n_cores=NCORE)


# ---------------------------------------------------------------- L1 node
def build_l1():
    nc = bacc.Bacc("TRN2", target_bir_lowering=False, debug=False, num_devices=NCORE)
    xT = nc.dram_tensor("xT", [128, NPAD], F32, kind="ExternalInput")
    W1 = nc.dram_tensor("W1", [128, HID], F32, kind="ExternalInput")
    avec = nc.dram_tensor("avec", [128, 2], F32, kind="ExternalInput")  # [as|ad] cols
    out = nc.dram_tensor("out", [PC, TCOLS], F32, kind="ExternalOutput")

    with tile.TileContext(nc) as tc:
        with (
            tc.tile_pool(name="c", bufs=1) as cp,
            tc.tile_pool(name="x", bufs=3) as xp,
            tc.tile_pool(name="o", bufs=3) as op,
            tc.tile_pool(name="ps", bufs=2, space="PSUM") as pp,
            tc.tile_pool(name="ps1", bufs=1, space="PSUM") as pp1,
        ):
            ident = cp.tile([128, 128], F32)
            make_identity(nc, ident[:])
            w_sb = cp.tile([128, HID], F32)
            nc.sync.dma_start(w_sb[:], W1[:])
            av_sb = cp.tile([128, 2], F32)
            nc.sync.dma_start(av_sb[:], avec[:])
            # W1T for v = W1 @ a
            wT_ps = pp1.tile([128, 128], F32, tag="tmp")
            nc.tensor.transpose(out=wT_ps[:], in_=w_sb[:], identity=ident[:])
            wT_sb = cp.tile([128, 128], F32)
            nc.vector.tensor_copy(out=wT_sb[:], in_=wT_ps[:])
            v_ps = pp1.tile([128, 2], F32, tag="tmp")
            nc.tensor.matmul(out=v_ps[:], lhsT=wT_sb[:], rhs=av_sb[:], start=True, stop=True)
            waug = cp.tile([128, HID + 2], F32)
            nc.vector.tensor_copy(out=waug[:, 0:HID], in_=w_sb[:])
            nc.vector.tensor_copy(out=waug[:, HID:HID + 2], in_=v_ps[:])
            for t in range(NBLK):
                xt = xp.tile([128, 128], F32, tag="x")
                nc.sync.dma_start(xt[:], xT[:, t * 128:(t + 1) * 128])
                h_ps = pp.tile([128, HID + 2], F32, tag="h")
                nc.tensor.matmul(out=h_ps[:], lhsT=xt[:], rhs=waug[:], start=True, stop=True)
                ot = op.tile([128, TCOLS], F32, tag="o")
                nc.vector.tensor_copy(out=ot[:, 0:HID + 2], in_=h_ps[:])
                nc.vector.memset(ot[:, HID + 2:HID + 3], 1.0)
                nc.vector.memset(ot[:, HID + 3:TCOLS], 0.0)
                nc.sync.dma_start(out[t * 128:(t + 1) * 128, :], ot[:])
    nc.compile()
    return nc


# ---------------------------------------------------------------- edge kernel
def build_edge(t_counts):
    """t_counts: list of NBLK subtile counts (shared across cores)."""
    nsub = int(sum(t_counts))
    nc = bacc.Bacc("TRN2", target_bir_lowering=False, debug=False, num_devices=NCORE)
    table = nc.dram_tensor("table", [NPAD, TCOLS], F32, kind="ExternalInput")
    src_idx = nc.dram_tensor("src_idx", [128, nsub], I32, kind="ExternalInput")
    dst_loc = nc.dram_tensor("dst_loc", [128, nsub], F32, kind="ExternalInput")
    agg = nc.dram_tensor("agg", [PC, HID], F32, kind="ExternalOutput")
    stats = nc.dram_tensor("stats", [1, 256], F32, kind="ExternalOutput")

    TMAX = max(t_counts)
    with tile.TileContext(nc) as tc:
        with (
            tc.tile_pool(name="c", bufs=1) as cp,
            tc.tile_pool(name="g", bufs=24) as gp,
            tc.tile_pool(name="s0", bufs=24) as s0p,
            tc.tile_pool(name="sw", bufs=3) as swp,
            tc.tile_pool(name="w", bufs=2) as wp,
            tc.tile_pool(name="ob", bufs=3) as obp,
            tc.tile_pool(name="own", bufs=2) as ownp,
            tc.tile_pool(name="pblk", bufs=2, space="PSUM") as pblk,
            tc.tile_pool(name="pal", bufs=2, space="PSUM") as pal,
            tc.tile_pool(name="ptr", bufs=2, space="PSUM") as ptr,
            tc.tile_pool(name="pst", bufs=1, space="PSUM") as pst,
        ):
            ident = cp.tile([128, 128], F32)
            make_identity(nc, ident[:])
            iota_i = cp.tile([128, 128], I32)
            nc.gpsimd.iota(iota_i[:], pattern=[[1, 128]], base=0, channel_multiplier=0)
            iota_f = cp.tile([128, 128], F32)
            nc.vector.tensor_copy(out=iota_f[:], in_=iota_i[:])
            ones_col = cp.tile([128, 1], F32)
            nc.vector.memset(ones_col[:], 1.0)
            idx_sb = cp.tile([128, nsub], I32)
            nc.sync.dma_start(idx_sb[:], src_idx[:])
            dl_sb = cp.tile([128, nsub], F32)
            nc.sync.dma_start(dl_sb[:], dst_loc[:])

            ps_sum = pst.tile([1, 128], F32, tag="sum")
            ps_sq = pst.tile([1, 128], F32, tag="sq")

            # zero-init gather buffers (stale-data guard)
            for i in range(24):
                gi = gp.tile([128, TCOLS], F32, tag="g", name=f"ginit{i}")
                nc.vector.memset(gi[:], 0.0)

            q0 = 0
            for t in range(NBLK):
                T = t_counts[t]
                # own rows: alpha_d of the block's nodes (cols 128..132)
                ownt = ownp.tile([128, 4], F32, tag="own")
                nc.sync.dma_start(ownt[:], table[t * 128:(t + 1) * 128, HID:HID + 4])
                ps_a = pal.tile([128, TMAX], F32, tag="al")
                s0_list = []
                g_list = []
                for s in range(T):
                    q = q0 + s
                    g = gp.tile([128, TCOLS], F32, tag="g")
                    nc.gpsimd.indirect_dma_start(
                        out=g[:], out_offset=None, in_=table[:],
                        in_offset=bass.IndirectOffsetOnAxis(ap=idx_sb[:, q:q + 1], axis=0))
                    g_list.append(g)
                    s0 = s0p.tile([128, 128], F32, tag="s0")
                    nc.vector.tensor_scalar(
                        out=s0[:], in0=iota_f[:], scalar1=dl_sb[:, q:q + 1],
                        scalar2=None, op0=mybir.AluOpType.is_equal)
                    s0_list.append(s0)
                    s0t_ps = ptr.tile([128, 128], F32, tag="tr")
                    nc.tensor.transpose(out=s0t_ps[:], in_=s0[:], identity=ident[:])
                    s0t = swp.tile([128, 128], F32, tag="s0t")
                    nc.vector.tensor_copy(out=s0t[:], in_=s0t_ps[:])
                    # alpha_d per edge -> ps_a[:, s]
                    nc.tensor.matmul(out=ps_a[:, s:s + 1], lhsT=s0t[:],
                                     rhs=ownt[:, 1:2], start=True, stop=True)
                # w = exp(lrelu(alpha_s + alpha_d)) for the whole block
                w_blk = wp.tile([128, TMAX], F32, tag="w")
                ps_b = pblk.tile([128, HID + 3], F32, tag="blk")
                for s in range(T):
                    g = g_list[s]
                    # alpha = alpha_s (g col 128) + alpha_d (ps_a col s)
                    nc.vector.tensor_tensor(
                        out=w_blk[:, s:s + 1], in0=g[:, HID:HID + 1],
                        in1=ps_a[:, s:s + 1], op=mybir.AluOpType.add)
                wb2 = wp.tile([128, TMAX], F32, tag="w2")
                nc.vector.tensor_scalar(out=wb2[:, 0:T], in0=w_blk[:, 0:T],
                                        scalar1=NEG, scalar2=None,
                                        op0=mybir.AluOpType.mult)
                nc.vector.tensor_tensor(out=w_blk[:, 0:T], in0=w_blk[:, 0:T],
                                        in1=wb2[:, 0:T], op=mybir.AluOpType.max)
                nc.scalar.activation(out=w_blk[:, 0:T], in_=w_blk[:, 0:T],
                                     func=mybir.ActivationFunctionType.Exp)
                for s in range(T):
                    sw = swp.tile([128, 128], F32, tag="sw")
                    nc.vector.tensor_scalar(
                        out=sw[:], in0=s0_list[s][:], scalar1=w_blk[:, s:s + 1],
                        scalar2=None, op0=mybir.AluOpType.mult)
                    nc.tensor.matmul(out=ps_b[:], lhsT=sw[:], rhs=g_list[s][:, 0:HID + 3],
                                     start=(s == 0), stop=(s == T - 1))
                # normalize: num = ps_b[:, 0:128], den = ps_b[:, 130]
                den = wp.tile([128, 1], F32, tag="den")
                nc.vector.tensor_scalar(out=den[:], in0=ps_b[:, HID + 2:HID + 3],
                                        scalar1=0.0, scalar2=None,
                                        op0=mybir.AluOpType.is_equal)
                nc.vector.tensor_tensor(out=den[:], in0=den[:],
                                        in1=ps_b[:, HID + 2:HID + 3],
                                        op=mybir.AluOpType.add)
                rec = wp.tile([128, 1], F32, tag="rec")
                nc.vector.reciprocal(out=rec[:], in_=den[:])
                ob = obp.tile([128, HID], F32, tag="ob")
                nc.vector.tensor_scalar(out=ob[:], in0=ps_b[:, 0:HID], scalar1=rec[:],
                                        scalar2=None, op0=mybir.AluOpType.mult)
                nc.sync.dma_start(agg[t * 128:(t + 1) * 128, :], ob[:])
                sq = obp.tile([128, HID], F32, tag="sq")
                nc.scalar.activation(out=sq[:], in_=ob[:],
                                     func=mybir.ActivationFunctionType.Square)
                nc.tensor.matmul(out=ps_sum[:], lhsT=ones_col[:], rhs=ob[:],
                                 start=(t == 0), stop=(t == NBLK - 1))
                nc.tensor.matmul(out=ps_sq[:], lhsT=ones_col[:], rhs=sq[:],
                                 start=(t == 0), stop=(t == NBLK - 1))
                q0 += T
            st_sb = cp.tile([1, 256], F32)
            nc.vector.tensor_copy(out=st_sb[:, 0:128], in_=ps_sum[:])
            nc.vector.tensor_copy(out=st_sb[:, 128:256], in_=ps_sq[:])
            nc.sync.dma_start(stats[:], st_sb[:])
    nc.compile()
    return nc


# ---------------------------------------------------------------- node tail
def build_node2(classifier):
    """BN apply + relu (+ next-layer table build, or classifier)."""
    nc = bacc.Bacc("TRN2", target_bir_lowering=False, debug=False, num_devices=NCORE)
    agg = nc.dram_tensor("agg", [PC, HID], F32, kind="ExternalInput")
    parts = nc.dram_tensor("parts", [8, 256], F32, kind="ExternalInput")
    gb = nc.dram_tensor("gb", [1, 256], F32, kind="ExternalInput")  # [gamma|beta]
    if classifier:
        Wn = nc.dram_tensor("Wn", [128, NCLS], F32, kind="ExternalInput")
        bc = nc.dram_tensor("bc", [1, NCLS], F32, kind="ExternalInput")
        out = nc.dram_tensor("out", [PC, NCLS], F32, kind="ExternalOutput")
    else:
        Wn = nc.dram_tensor("Wn", [128, HID], F32, kind="ExternalInput")
        avec = nc.dram_tensor("avec", [128, 2], F32, kind="ExternalInput")
        out = nc.dram_tensor("out", [PC, TCOLS], F32, kind="ExternalOutput")

    with tile.TileContext(nc) as tc:
        with (
            tc.tile_pool(name="c", bufs=1) as cp,
            tc.tile_pool(name="x", bufs=3) as xp,
            tc.tile_pool(name="o", bufs=3) as op,
            tc.tile_pool(name="ps", bufs=2, space="PSUM") as pp,
            tc.tile_pool(name="ps1", bufs=2, space="PSUM") as pp1,
        ):
            ident = cp.tile([128, 128], F32)
            make_identity(nc, ident[:])
            parts_sb = cp.tile([8, 256], F32)
            nc.sync.dma_start(parts_sb[:], parts[:])
            ones8 = cp.tile([8, 1], F32)
            nc.vector.memset(ones8[:], 1.0)
            st_ps = pp1.tile([1, 256], F32, tag="tmp")
            nc.tensor.matmul(out=st_ps[:], lhsT=ones8[:], rhs=parts_sb[:], start=True, stop=True)
            stat = cp.tile([1, 256], F32)
            nc.vector.tensor_scalar(out=stat[:], in0=st_ps[:], scalar1=1.0 / N,
                                    scalar2=None, op0=mybir.AluOpType.mult)
            mean = stat[:, 0:128]
            msq = stat[:, 128:256]
            var = cp.tile([1, 128], F32)
            nc.vector.tensor_tensor(out=var[:], in0=mean, in1=mean, op=mybir.AluOpType.mult)
            nc.vector.tensor_tensor(out=var[:], in0=msq, in1=var[:], op=mybir.AluOpType.subtract)
            nc.vector.tensor_scalar(out=var[:], in0=var[:], scalar1=EPS,
                                    scalar2=None, op0=mybir.AluOpType.add)
            std = cp.tile([1, 128], F32)
            nc.scalar.activation(out=std[:], in_=var[:],
                                 func=mybir.ActivationFunctionType.Sqrt)
            istd = cp.tile([1, 128], F32)
            nc.vector.reciprocal(out=istd[:], in_=std[:])
            gb_sb = cp.tile([1, 256], F32)
            nc.sync.dma_start(gb_sb[:], gb[:])
            gam = cp.tile([1, 128], F32)
            nc.vector.tensor_tensor(out=gam[:], in0=gb_sb[:, 0:128], in1=istd[:],
                                    op=mybir.AluOpType.mult)
            bet = cp.tile([1, 128], F32)
            nc.vector.tensor_tensor(out=bet[:], in0=mean, in1=gam[:], op=mybir.AluOpType.mult)
            nc.vector.tensor_tensor(out=bet[:], in0=gb_sb[:, 128:256], in1=bet[:],
                                    op=mybir.AluOpType.subtract)
            # broadcast gamma', beta' to [128, 128] via K=1 matmul
            one1 = cp.tile([1, 128], F32)
            nc.vector.memset(one1[:], 1.0)
            gbc_ps = pp1.tile([128, 128], F32, tag="tmp")
            nc.tensor.matmul(out=gbc_ps[:], lhsT=one1[:], rhs=gam[:], start=True, stop=True)
            gbc = cp.tile([128, 128], F32)
            nc.vector.tensor_copy(out=gbc[:], in_=gbc_ps[:])
            bbc_ps = pp1.tile([128, 128], F32, tag="tmp")
            nc.tensor.matmul(out=bbc_ps[:], lhsT=one1[:], rhs=bet[:], start=True, stop=True)
            bbc = cp.tile([128, 128], F32)
            nc.vector.tensor_copy(out=bbc[:], in_=bbc_ps[:])

            if classifier:
                wn_sb = cp.tile([128, NCLS], F32)
                nc.sync.dma_start(wn_sb[:], Wn[:])
                bc_sb = cp.tile([1, NCLS], F32)
                nc.sync.dma_start(bc_sb[:], bc[:])
                bcb_ps = pp1.tile([128, NCLS], F32, tag="tmp")
                nc.tensor.matmul(out=bcb_ps[:], lhsT=one1[:], rhs=bc_sb[:], start=True, stop=True)
                bcb = cp.tile([128, NCLS], F32)
                nc.vector.tensor_copy(out=bcb[:], in_=bcb_ps[:])
                rhs_w = wn_sb
                ncols = NCLS
            else:
                wn_sb = cp.tile([128, HID], F32)
                nc.sync.dma_start(wn_sb[:], Wn[:])
                av_sb = cp.tile([128, 2], F32)
                nc.sync.dma_start(av_sb[:], avec[:])
                wT_ps = pp1.tile([128, 128], F32, tag="tmp")
                nc.tensor.transpose(out=wT_ps[:], in_=wn_sb[:], identity=ident[:])
                wT_sb = cp.tile([128, 128], F32)
                nc.vector.tensor_copy(out=wT_sb[:], in_=wT_ps[:])
                v_ps = pp1.tile([128, 2], F32, tag="tmp")
                nc.tensor.matmul(out=v_ps[:], lhsT=wT_sb[:], rhs=av_sb[:], start=True, stop=True)
                waug = cp.tile([128, HID + 2], F32)
                nc.vector.tensor_copy(out=waug[:, 0:HID], in_=wn_sb[:])
                nc.vector.tensor_copy(out=waug[:, HID:HID + 2], in_=v_ps[:])
                rhs_w = waug
                ncols = HID + 2

            for t in range(NBLK):
                at = xp.tile([128, HID], F32, tag="a")
                nc.sync.dma_start(at[:], agg[t * 128:(t + 1) * 128, :])
                x2 = xp.tile([128, HID], F32, tag="x2")
                nc.vector.tensor_tensor(out=x2[:], in0=at[:], in1=gbc[:], op=mybir.AluOpType.mult)
                nc.vector.tensor_tensor(out=x2[:], in0=x2[:], in1=bbc[:], op=mybir.AluOpType.add)
                nc.scalar.activation(out=x2[:], in_=x2[:],
                                     func=mybir.ActivationFunctionType.Relu)
                xT_ps = pp.tile([128, 128], F32, tag="xt")
                nc.tensor.transpose(out=xT_ps[:], in_=x2[:], identity=ident[:])
                xT_sb = xp.tile([128, 128], F32, tag="xts")
                nc.vector.tensor_copy(out=xT_sb[:], in_=xT_ps[:])
                h_ps = pp.tile([128, ncols], F32, tag="h")
                nc.tensor.matmul(out=h_ps[:], lhsT=xT_sb[:], rhs=rhs_w[:], start=True, stop=True)
                if classifier:
                    ot = op.tile([128, NCLS], F32, tag="o")
                    nc.vector.tensor_tensor(out=ot[:], in0=h_ps[:], in1=bcb[:],
                                            op=mybir.AluOpType.add)
                    nc.sync.dma_start(out[t * 128:(t + 1) * 128, :], ot[:])
                else:
                    ot = op.tile([128, TCOLS], F32, tag="o")
                    nc.vector.tensor_copy(out=ot[:, 0:HID + 2], in_=h_ps[:])
                    nc.vector.memset(ot[:, HID + 2:HID + 3], 1.0)
                    nc.vector.memset(ot[:, HID + 3:TCOLS], 0.0)
                    nc.sync.dma_start(out[t * 128:(t + 1) * 128, :], ot[:])
    nc.compile()
    return nc


# ---------------------------------------------------------------- host glue
def _edge_arrays(src, dst):
    """Build per-core src_idx/dst_local arrays + shared t_counts."""
    order = np.argsort(dst, kind="stable")
    srcs = src[order]
    dsts = dst[order]
    blk = (dsts // BLK).astype(np.int64)
    counts = np.bincount(blk, minlength=NPAD // BLK)
    starts = np.concatenate([[0], np.cumsum(counts)])
    # shared subtile counts per slot t: max over cores
    cnt_mat = counts.reshape(NCORE, NBLK)
    t_counts = np.maximum(np.ceil(cnt_mat / BLK).astype(np.int64).max(axis=0), 1)
    nsub = int(t_counts.sum())
    offs = np.concatenate([[0], np.cumsum(t_counts)])
    src_arrs, dst_arrs = [], []
    for c in range(NCORE):
        si = np.zeros((128, nsub), np.int32)
        dl = np.full((128, nsub), 200.0, np.float32)
        for t in range(NBLK):
            b = c * NBLK + t
            s0, e0 = starts[b], starts[b + 1]
            cnt = e0 - s0
            if cnt == 0:
                continue
            k = np.arange(cnt)
            p = k % 128
            q = offs[t] + k // 128
            rolled = (srcs[s0:e0] - PC * c) % NPAD
            si[p, q] = rolled.astype(np.int32)
            dl[p, q] = (dsts[s0:e0] - b * BLK).astype(np.float32)
        src_arrs.append(si)
        dst_arrs.append(dl)
    return t_counts, src_arrs, dst_arrs


_CACHE = {}


def kernel(x, edge_index, W1, as1, ad1, b1, g1, beta1,
           W2, as2, ad2, b2, g2, beta2, Wc, bc):
    x = np.asarray(x, np.float32)
    ei = np.asarray(edge_index)
    src = np.concatenate([ei[0], np.arange(N, dtype=ei.dtype)]).astype(np.int64)
    dst = np.concatenate([ei[1], np.arange(N, dtype=ei.dtype)]).astype(np.int64)

    t_counts, src_arrs, dst_arrs = _edge_arrays(src, dst)

    key = tuple(t_counts.tolist())
    if key not in _CACHE:
        _CACHE[key] = (build_l1(), build_edge(t_counts),
                       build_node2(False), build_node2(True))
    nc1, nce, nc3, nc5 = _CACHE[key]

    # ---- L1
    xT = np.zeros((128, NPAD), np.float32)
    xT[:, :N] = np.asarray(x, np.float32).T
    av = np.stack([np.asarray(as1, np.float32), np.asarray(ad1, np.float32)], axis=1)
    in1 = [{"xT": np.roll(xT, -PC * c, axis=1).copy(),
            "W1": np.asarray(W1, np.float32), "avec": av} for c in range(NCORE)]
    r1 = _run(nc1, in1, "L1")
    h1 = np.concatenate([r1[c]["out"] for c in range(NCORE)], axis=0)  # [NPAD, 132]

    # ---- E1
    ine = [{"table": np.roll(h1, -PC * c, axis=0).copy(),
            "src_idx": src_arrs[c], "dst_loc": dst_arrs[c]} for c in range(NCORE)]
    re1 = _run(nce, ine, "E1")
    agg1 = [re1[c]["agg"] for c in range(NCORE)]
    parts1 = np.stack([re1[c]["stats"][0] for c in range(NCORE)], axis=0)  # [8, 256]

    # ---- L3
    gb1 = np.concatenate([np.asarray(g1, np.float32),
                          np.asarray(beta1, np.float32)])[None, :]
    av2 = np.stack([np.asarray(as2, np.float32), np.asarray(ad2, np.float32)], axis=1)
    in3 = [{"agg": agg1[c], "parts": parts1, "gb": gb1,
            "Wn": np.asarray(W2, np.float32), "avec": av2} for c in range(NCORE)]
    r3 = _run(nc3, in3, "L3")
    h2 = np.concatenate([r3[c]["out"] for c in range(NCORE)], axis=0)

    # ---- E2
    ine2 = [{"table": np.roll(h2, -PC * c, axis=0).copy(),
             "src_idx": src_arrs[c], "dst_loc": dst_arrs[c]} for c in range(NCORE)]
    re2 = _run(nce, ine2, "E2")
    agg2 = [re2[c]["agg"] for c in range(NCORE)]
    parts2 = np.stack([re2[c]["stats"][0] for c in range(NCORE)], axis=0)

    # ---- L5
    gb2 = np.concatenate([np.asarray(g2, np.float32),
                          np.asarray(beta2, np.float32)])[None, :]
    in5 = [{"agg": agg2[c], "parts": parts2, "gb": gb2,
            "Wn": np.asarray(Wc, np.float32),
            "bc": np.asarray(bc, np.float32)[None, :]} for c in range(NCORE)]
    r5 = _run(nc5, in5, "L5")
    logits = np.concatenate([r5[c]["out"] for c in range(NCORE)], axis=0)
    return logits[:N]


# revision 13
# speedup vs baseline: 1.2457x; 1.0023x over previous
"""GAT 2-layer + BN + classifier on 8 TRN2 NeuronCores (Bass/Tile).

Strategy: dst-block sharding with per-core table rotation so the SPMD
instruction stream is identical across cores. 5 launches:
  L1   node: h1_aug = x @ [W1 | W1@as1 | W1@ad1], table rows [h|as|ad|1|0]
  E(1) edge: gather h1_aug[src] per edge subtile, segment softmax via
       selection-matrix matmul in PSUM, partial BN stats
  L3   node: BN1 apply + relu + @W2_aug -> h2_aug table shard
  E(2) edge: same NEFF as E(1) on h2_aug
  L5   node: BN2 apply + relu + @Wc + bc -> logits shard
Host work is index-only: edge sort/shard, table assembly/rotation.
"""
import sys
sys.path.insert(0, '/opt/trn_rl_repo')
sys.path.insert(0, '/root/.axon_site')
import numpy as np

import concourse.bass as bass
import concourse.bacc as bacc
import concourse.tile as tile
from concourse import mybir
from concourse.masks import make_identity

F32 = mybir.dt.float32
I32 = mybir.dt.int32

N = 100000
NCORE = 8
BLK = 128
NPAD = 100352            # 784 blocks of 128
PC = NPAD // NCORE       # 12544 nodes/core = 98 blocks
NBLK = PC // BLK         # 98
TCOLS = 132              # table row: [h(128) | as | ad | one | pad]
HID = 128
NCLS = 40
NEG = 0.2
EPS = 1e-5

_EXEC_NS = []            # per-launch exec times when profiling enabled
PROFILE = False


RUN_HOOK = None          # test harness may set this to a profiling runner


def _run(nc, in_maps, label):
    if RUN_HOOK is not None:
        return RUN_HOOK(nc, in_maps, label)
    from concourse import bass2jax
    return bass2jax.run_bass_via_pjrt(nc, in_maps, n_cores=NCORE)


# ---------------------------------------------------------------- L1 node
def build_l1():
    nc = bacc.Bacc("TRN2", target_bir_lowering=False, debug=False, num_devices=NCORE)
    xT = nc.dram_tensor("xT", [128, NPAD], F32, kind="ExternalInput")
    W1 = nc.dram_tensor("W1", [128, HID], F32, kind="ExternalInput")
    avec = nc.dram_tensor("avec", [128, 2], F32, kind="ExternalInput")  # [as|ad] cols
    out = nc.dram_tensor("out", [PC, TCOLS], F32, kind="ExternalOutput")

    with tile.TileContext(nc) as tc:
        with (
            tc.tile_pool(name="c", bufs=1) as cp,
            tc.tile_pool(name="x", bufs=3) as xp,
            tc.tile_pool(name="o", bufs=3) as op,
            tc.tile_pool(name="ps", bufs=2, space="PSUM") as pp,
            tc.tile_pool(name="ps1", bufs=1, space="PSUM") as pp1,
        ):
            ident = cp.tile([128, 128], F32)
            make_identity(nc, ident[:])
            w_sb = cp.tile([128, HID], F32)
            nc.sync.dma_start(w_sb[:], W1[:])
            av_sb = cp.tile([128, 2], F32)
            nc.sync.dma_start(av_sb[:], avec[:])
            # W1T for v = W1 @ a
            wT_ps = pp1.tile([128, 128], F32, tag="tmp")
            nc.tensor.transpose(out=wT_ps[:], in_=w_sb[:], identity=ident[:])
            wT_sb = cp.tile([128, 128], F32)
            nc.vector.tensor_copy(out=wT_sb[:], in_=wT_ps[:])
            v_ps = pp1.tile([128, 2], F32, tag="tmp")
            nc.tensor.matmul(out=v_ps[:], lhsT=wT_sb[:], rhs=av_sb[:], start=True, stop=True)
            waug = cp.tile([128, HID + 2], F32)
            nc.vector.tensor_copy(out=waug[:, 0:HID], in_=w_sb[:])
            nc.vector.tensor_copy(out=waug[:, HID:HID + 2], in_=v_ps[:])
            GB = 4
            for g0 in range(0, NBLK, GB):
                nb = min(GB, NBLK - g0)
                xs = xp.tile([128, GB * 128], F32, tag="x", name=f"xs{g0}")
                nc.sync.dma_start(xs[:, 0:nb * 128],
                                  xT[:, g0 * 128:(g0 + nb) * 128])
                for i in range(nb):
                    t = g0 + i
                    h_ps = pp.tile([128, HID + 2], F32, tag="h", name=f"h{t}")
                    nc.tensor.matmul(out=h_ps[:], lhsT=xs[:, i * 128:(i + 1) * 128],
                                     rhs=waug[:], start=True, stop=True)
                    ot = op.tile([128, TCOLS], F32, tag="o", name=f"o{t}")
                    nc.vector.tensor_copy(out=ot[:, 0:HID + 2], in_=h_ps[:])
                    nc.vector.memset(ot[:, HID + 2:HID + 3], 1.0)
                    nc.vector.memset(ot[:, HID + 3:TCOLS], 0.0)
                    nc.scalar.dma_start(out[t * 128:(t + 1) * 128, :], ot[:])
    nc.compile()
    return nc


# ---------------------------------------------------------------- edge kernel
def build_edge(t_counts):
    """t_counts: list of NBLK subtile counts (shared across cores)."""
    nsub = int(sum(t_counts))
    nc = bacc.Bacc("TRN2", target_bir_lowering=False, debug=False, num_devices=NCORE)
    table = nc.dram_tensor("table", [NPAD, TCOLS], F32, kind="ExternalInput")
    src_idx = nc.dram_tensor("src_idx", [128, nsub], I32, kind="ExternalInput")
    dst_loc = nc.dram_tensor("dst_loc", [128, nsub], F32, kind="ExternalInput")
    agg = nc.dram_tensor("agg", [PC, HID], F32, kind="ExternalOutput")
    stats = nc.dram_tensor("stats", [1, 256], F32, kind="ExternalOutput")

    TMAX = max(t_counts)
    with tile.TileContext(nc) as tc:
        with (
            tc.tile_pool(name="c", bufs=1) as cp,
            tc.tile_pool(name="g", bufs=24) as gp,
            tc.tile_pool(name="s0", bufs=24) as s0p,
            tc.tile_pool(name="sw", bufs=3) as swp,
            tc.tile_pool(name="w", bufs=2) as wp,
            tc.tile_pool(name="ob", bufs=3) as obp,
            tc.tile_pool(name="own", bufs=2) as ownp,
            tc.tile_pool(name="pblk", bufs=2, space="PSUM") as pblk,
            tc.tile_pool(name="pal", bufs=2, space="PSUM") as pal,
            tc.tile_pool(name="ptr", bufs=2, space="PSUM") as ptr,
            tc.tile_pool(name="pst", bufs=1, space="PSUM") as pst,
        ):
            ident = cp.tile([128, 128], F32)
            make_identity(nc, ident[:])
            iota_i = cp.tile([128, 128], I32)
            nc.gpsimd.iota(iota_i[:], pattern=[[1, 128]], base=0, channel_multiplier=0)
            iota_f = cp.tile([128, 128], F32)
            nc.vector.tensor_copy(out=iota_f[:], in_=iota_i[:])
            ones_col = cp.tile([128, 1], F32)
            nc.vector.memset(ones_col[:], 1.0)
            idx_sb = cp.tile([128, nsub], I32)
            nc.sync.dma_start(idx_sb[:], src_idx[:])
            dl_sb = cp.tile([128, nsub], F32)
            nc.sync.dma_start(dl_sb[:], dst_loc[:])

            ps_sum = pst.tile([1, 128], F32, tag="sum")
            ps_sq = pst.tile([1, 128], F32, tag="sq")

            # zero-init gather buffers (stale-data guard)
            for i in range(24):
                gi = gp.tile([128, TCOLS], F32, tag="g", name=f"ginit{i}")
                nc.vector.memset(gi[:], 0.0)

            q0 = 0
            for t in range(NBLK):
                T = t_counts[t]
                # own rows: alpha_d of the block's nodes (cols 128..132)
                ownt = ownp.tile([128, 4], F32, tag="own")
                nc.sync.dma_start(ownt[:], table[t * 128:(t + 1) * 128, HID:HID + 4])
                ps_a = pal.tile([128, TMAX], F32, tag="al")
                s0_list = []
                g_list = []
                for s in range(T):
                    q = q0 + s
                    g = gp.tile([128, TCOLS], F32, tag="g")
                    nc.gpsimd.indirect_dma_start(
                        out=g[:], out_offset=None, in_=table[:],
                        in_offset=bass.IndirectOffsetOnAxis(ap=idx_sb[:, q:q + 1], axis=0))
                    g_list.append(g)
                    s0 = s0p.tile([128, 128], F32, tag="s0")
                    nc.vector.tensor_scalar(
                        out=s0[:], in0=iota_f[:], scalar1=dl_sb[:, q:q + 1],
                        scalar2=None, op0=mybir.AluOpType.is_equal)
                    s0_list.append(s0)
                    s0t_ps = ptr.tile([128, 128], F32, tag="tr")
                    nc.tensor.transpose(out=s0t_ps[:], in_=s0[:], identity=ident[:])
                    s0t = swp.tile([128, 128], F32, tag="s0t")
                    nc.vector.tensor_copy(out=s0t[:], in_=s0t_ps[:])
                    # alpha_d per edge -> ps_a[:, s]
                    nc.tensor.matmul(out=ps_a[:, s:s + 1], lhsT=s0t[:],
                                     rhs=ownt[:, 1:2], start=True, stop=True)
                # w = exp(lrelu(alpha_s + alpha_d)) for the whole block
                w_blk = wp.tile([128, TMAX], F32, tag="w")
                ps_b = pblk.tile([128, HID + 3], F32, tag="blk")
                for s in range(T):
                    g = g_list[s]
                    # alpha = alpha_s (g col 128) + alpha_d (ps_a col s)
                    nc.vector.tensor_tensor(
                        out=w_blk[:, s:s + 1], in0=g[:, HID:HID + 1],
                        in1=ps_a[:, s:s + 1], op=mybir.AluOpType.add)
                wb2 = wp.tile([128, TMAX], F32, tag="w2")
                nc.vector.tensor_scalar(out=wb2[:, 0:T], in0=w_blk[:, 0:T],
                                        scalar1=NEG, scalar2=None,
                                        op0=mybir.AluOpType.mult)
                nc.vector.tensor_tensor(out=w_blk[:, 0:T], in0=w_blk[:, 0:T],
                                        in1=wb2[:, 0:T], op=mybir.AluOpType.max)
                nc.scalar.activation(out=w_blk[:, 0:T], in_=w_blk[:, 0:T],
                                     func=mybir.ActivationFunctionType.Exp)
                for s in range(T):
                    sw = swp.tile([128, 128], F32, tag="sw")
                    nc.vector.tensor_scalar(
                        out=sw[:], in0=s0_list[s][:], scalar1=w_blk[:, s:s + 1],
                        scalar2=None, op0=mybir.AluOpType.mult)
                    nc.tensor.matmul(out=ps_b[:], lhsT=sw[:], rhs=g_list[s][:, 0:HID + 3],
                                     start=(s == 0), stop=(s == T - 1))
                # normalize: num = ps_b[:, 0:128], den = ps_b[:, 130]
                den = wp.tile([128, 1], F32, tag="den")
                nc.vector.tensor_scalar(out=den[:], in0=ps_b[:, HID + 2:HID + 3],
                                        scalar1=0.0, scalar2=None,
                                        op0=mybir.AluOpType.is_equal)
                nc.vector.tensor_tensor(out=den[:], in0=den[:],
                                        in1=ps_b[:, HID + 2:HID + 3],
                                        op=mybir.AluOpType.add)
                rec = wp.tile([128, 1], F32, tag="rec")
                nc.vector.reciprocal(out=rec[:], in_=den[:])
                ob = obp.tile([128, HID], F32, tag="ob")
                nc.vector.tensor_scalar(out=ob[:], in0=ps_b[:, 0:HID], scalar1=rec[:],
                                        scalar2=None, op0=mybir.AluOpType.mult)
                nc.sync.dma_start(agg[t * 128:(t + 1) * 128, :], ob[:])
                sq = obp.tile([128, HID], F32, tag="sq")
                nc.scalar.activation(out=sq[:], in_=ob[:],
                                     func=mybir.ActivationFunctionType.Square)
                nc.tensor.matmul(out=ps_sum[:], lhsT=ones_col[:], rhs=ob[:],
                                 start=(t == 0), stop=(t == NBLK - 1))
                nc.tensor.matmul(out=ps_sq[:], lhsT=ones_col[:], rhs=sq[:],
                                 start=(t == 0), stop=(t == NBLK - 1))
                q0 += T
            st_sb = cp.tile([1, 256], F32)
            nc.vector.tensor_copy(out=st_sb[:, 0:128], in_=ps_sum[:])
            nc.vector.tensor_copy(out=st_sb[:, 128:256], in_=ps_sq[:])
            nc.sync.dma_start(stats[:], st_sb[:])
    nc.compile()
    return nc


# ---------------------------------------------------------------- node tail
def build_node2(classifier):
    """BN apply + relu (+ next-layer table build, or classifier)."""
    nc = bacc.Bacc("TRN2", target_bir_lowering=False, debug=False, num_devices=NCORE)
    agg = nc.dram_tensor("agg", [PC, HID], F32, kind="ExternalInput")
    parts = nc.dram_tensor("parts", [8, 256], F32, kind="ExternalInput")
    gb = nc.dram_tensor("gb", [1, 256], F32, kind="ExternalInput")  # [gamma|beta]
    if classifier:
        Wn = nc.dram_tensor("Wn", [128, NCLS], F32, kind="ExternalInput")
        bc = nc.dram_tensor("bc", [1, NCLS], F32, kind="ExternalInput")
        out = nc.dram_tensor("out", [PC, NCLS], F32, kind="ExternalOutput")
    else:
        Wn = nc.dram_tensor("Wn", [128, HID], F32, kind="ExternalInput")
        avec = nc.dram_tensor("avec", [128, 2], F32, kind="ExternalInput")
        out = nc.dram_tensor("out", [PC, TCOLS], F32, kind="ExternalOutput")

    with tile.TileContext(nc) as tc:
        with (
            tc.tile_pool(name="c", bufs=1) as cp,
            tc.tile_pool(name="x", bufs=3) as xp,
            tc.tile_pool(name="o", bufs=3) as op,
            tc.tile_pool(name="ps", bufs=2, space="PSUM") as pp,
            tc.tile_pool(name="ps1", bufs=2, space="PSUM") as pp1,
        ):
            ident = cp.tile([128, 128], F32)
            make_identity(nc, ident[:])
            parts_sb = cp.tile([8, 256], F32)
            nc.sync.dma_start(parts_sb[:], parts[:])
            ones8 = cp.tile([8, 1], F32)
            nc.vector.memset(ones8[:], 1.0)
            st_ps = pp1.tile([1, 256], F32, tag="tmp")
            nc.tensor.matmul(out=st_ps[:], lhsT=ones8[:], rhs=parts_sb[:], start=True, stop=True)
            stat = cp.tile([1, 256], F32)
            nc.vector.tensor_scalar(out=stat[:], in0=st_ps[:], scalar1=1.0 / N,
                                    scalar2=None, op0=mybir.AluOpType.mult)
            mean = stat[:, 0:128]
            msq = stat[:, 128:256]
            var = cp.tile([1, 128], F32)
            nc.vector.tensor_tensor(out=var[:], in0=mean, in1=mean, op=mybir.AluOpType.mult)
            nc.vector.tensor_tensor(out=var[:], in0=msq, in1=var[:], op=mybir.AluOpType.subtract)
            nc.vector.tensor_scalar(out=var[:], in0=var[:], scalar1=EPS,
                                    scalar2=None, op0=mybir.AluOpType.add)
            std = cp.tile([1, 128], F32)
            nc.scalar.activation(out=std[:], in_=var[:],
                                 func=mybir.ActivationFunctionType.Sqrt)
            istd = cp.tile([1, 128], F32)
            nc.vector.reciprocal(out=istd[:], in_=std[:])
            gb_sb = cp.tile([1, 256], F32)
            nc.sync.dma_start(gb_sb[:], gb[:])
            gam = cp.tile([1, 128], F32)
            nc.vector.tensor_tensor(out=gam[:], in0=gb_sb[:, 0:128], in1=istd[:],
                                    op=mybir.AluOpType.mult)
            bet = cp.tile([1, 128], F32)
            nc.vector.tensor_tensor(out=bet[:], in0=mean, in1=gam[:], op=mybir.AluOpType.mult)
            nc.vector.tensor_tensor(out=bet[:], in0=gb_sb[:, 128:256], in1=bet[:],
                                    op=mybir.AluOpType.subtract)
            # broadcast gamma', beta' to [128, 128] via K=1 matmul
            one1 = cp.tile([1, 128], F32)
            nc.vector.memset(one1[:], 1.0)
            gbc_ps = pp1.tile([128, 128], F32, tag="tmp")
            nc.tensor.matmul(out=gbc_ps[:], lhsT=one1[:], rhs=gam[:], start=True, stop=True)
            gbc = cp.tile([128, 128], F32)
            nc.vector.tensor_copy(out=gbc[:], in_=gbc_ps[:])
            bbc_ps = pp1.tile([128, 128], F32, tag="tmp")
            nc.tensor.matmul(out=bbc_ps[:], lhsT=one1[:], rhs=bet[:], start=True, stop=True)
            bbc = cp.tile([128, 128], F32)
            nc.vector.tensor_copy(out=bbc[:], in_=bbc_ps[:])

            if classifier:
                wn_sb = cp.tile([128, NCLS], F32)
                nc.sync.dma_start(wn_sb[:], Wn[:])
                bc_sb = cp.tile([1, NCLS], F32)
                nc.sync.dma_start(bc_sb[:], bc[:])
                bcb_ps = pp1.tile([128, NCLS], F32, tag="tmp")
                nc.tensor.matmul(out=bcb_ps[:], lhsT=one1[:], rhs=bc_sb[:], start=True, stop=True)
                bcb = cp.tile([128, NCLS], F32)
                nc.vector.tensor_copy(out=bcb[:], in_=bcb_ps[:])
                rhs_w = wn_sb
                ncols = NCLS
            else:
                wn_sb = cp.tile([128, HID], F32)
                nc.sync.dma_start(wn_sb[:], Wn[:])
                av_sb = cp.tile([128, 2], F32)
                nc.sync.dma_start(av_sb[:], avec[:])
                wT_ps = pp1.tile([128, 128], F32, tag="tmp")
                nc.tensor.transpose(out=wT_ps[:], in_=wn_sb[:], identity=ident[:])
                wT_sb = cp.tile([128, 128], F32)
                nc.vector.tensor_copy(out=wT_sb[:], in_=wT_ps[:])
                v_ps = pp1.tile([128, 2], F32, tag="tmp")
                nc.tensor.matmul(out=v_ps[:], lhsT=wT_sb[:], rhs=av_sb[:], start=True, stop=True)
                waug = cp.tile([128, HID + 2], F32)
                nc.vector.tensor_copy(out=waug[:, 0:HID], in_=wn_sb[:])
                nc.vector.tensor_copy(out=waug[:, HID:HID + 2], in_=v_ps[:])
                rhs_w = waug
                ncols = HID + 2

            for t in range(NBLK):
                at = xp.tile([128, HID], F32, tag="a")
                nc.sync.dma_start(at[:], agg[t * 128:(t + 1) * 128, :])
                x2 = xp.tile([128, HID], F32, tag="x2")
                nc.vector.tensor_tensor(out=x2[:], in0=at[:], in1=gbc[:], op=mybir.AluOpType.mult)
                nc.vector.tensor_tensor(out=x2[:], in0=x2[:], in1=bbc[:], op=mybir.AluOpType.add)
                nc.scalar.activation(out=x2[:], in_=x2[:],
                                     func=mybir.ActivationFunctionType.Relu)
                xT_ps = pp.tile([128, 128], F32, tag="xt")
                nc.tensor.transpose(out=xT_ps[:], in_=x2[:], identity=ident[:])
                xT_sb = xp.tile([128, 128], F32, tag="xts")
                nc.vector.tensor_copy(out=xT_sb[:], in_=xT_ps[:])
                h_ps = pp.tile([128, ncols], F32, tag="h")
                nc.tensor.matmul(out=h_ps[:], lhsT=xT_sb[:], rhs=rhs_w[:], start=True, stop=True)
                if classifier:
                    ot = op.tile([128, NCLS], F32, tag="o")
                    nc.vector.tensor_tensor(out=ot[:], in0=h_ps[:], in1=bcb[:],
                                            op=mybir.AluOpType.add)
                    nc.scalar.dma_start(out[t * 128:(t + 1) * 128, :], ot[:])
                else:
                    ot = op.tile([128, TCOLS], F32, tag="o")
                    nc.vector.tensor_copy(out=ot[:, 0:HID + 2], in_=h_ps[:])
                    nc.vector.memset(ot[:, HID + 2:HID + 3], 1.0)
                    nc.vector.memset(ot[:, HID + 3:TCOLS], 0.0)
                    nc.scalar.dma_start(out[t * 128:(t + 1) * 128, :], ot[:])
    nc.compile()
    return nc


# ---------------------------------------------------------------- host glue
def _edge_arrays(src, dst):
    """Build per-core src_idx/dst_local arrays + shared t_counts."""
    order = np.argsort(dst, kind="stable")
    srcs = src[order]
    dsts = dst[order]
    blk = (dsts // BLK).astype(np.int64)
    counts = np.bincount(blk, minlength=NPAD // BLK)
    starts = np.concatenate([[0], np.cumsum(counts)])
    # shared subtile counts per slot t: max over cores
    cnt_mat = counts.reshape(NCORE, NBLK)
    t_counts = np.maximum(np.ceil(cnt_mat / BLK).astype(np.int64).max(axis=0), 1)
    nsub = int(t_counts.sum())
    offs = np.concatenate([[0], np.cumsum(t_counts)])
    src_arrs, dst_arrs = [], []
    for c in range(NCORE):
        si = np.zeros((128, nsub), np.int32)
        dl = np.full((128, nsub), 200.0, np.float32)
        for t in range(NBLK):
            b = c * NBLK + t
            s0, e0 = starts[b], starts[b + 1]
            cnt = e0 - s0
            if cnt == 0:
                continue
            k = np.arange(cnt)
            p = k % 128
            q = offs[t] + k // 128
            rolled = (srcs[s0:e0] - PC * c) % NPAD
            si[p, q] = rolled.astype(np.int32)
            dl[p, q] = (dsts[s0:e0] - b * BLK).astype(np.float32)
        src_arrs.append(si)
        dst_arrs.append(dl)
    return t_counts, src_arrs, dst_arrs


_CACHE = {}


def kernel(x, edge_index, W1, as1, ad1, b1, g1, beta1,
           W2, as2, ad2, b2, g2, beta2, Wc, bc):
    x = np.asarray(x, np.float32)
    ei = np.asarray(edge_index)
    src = np.concatenate([ei[0], np.arange(N, dtype=ei.dtype)]).astype(np.int64)
    dst = np.concatenate([ei[1], np.arange(N, dtype=ei.dtype)]).astype(np.int64)

    t_counts, src_arrs, dst_arrs = _edge_arrays(src, dst)

    key = tuple(t_counts.tolist())
    if key not in _CACHE:
        _CACHE[key] = (build_l1(), build_edge(t_counts),
                       build_node2(False), build_node2(True))
    nc1, nce, nc3, nc5 = _CACHE[key]

    # ---- L1
    xT = np.zeros((128, NPAD), np.float32)
    xT[:, :N] = np.asarray(x, np.float32).T
    av = np.stack([np.asarray(as1, np.float32), np.asarray(ad1, np.float32)], axis=1)
    in1 = [{"xT": np.roll(xT, -PC * c, axis=1).copy(),
            "W1": np.asarray(W1, np.float32), "avec": av} for c in range(NCORE)]
    r1 = _run(nc1, in1, "L1")
    h1 = np.concatenate([r1[c]["out"] for c in range(NCORE)], axis=0)  # [NPAD, 132]

    # ---- E1
    ine = [{"table": np.roll(h1, -PC * c, axis=0).copy(),
            "src_idx": src_arrs[c], "dst_loc": dst_arrs[c]} for c in range(NCORE)]
    re1 = _run(nce, ine, "E1")
    agg1 = [re1[c]["agg"] for c in range(NCORE)]
    parts1 = np.stack([re1[c]["stats"][0] for c in range(NCORE)], axis=0)  # [8, 256]

    # ---- L3
    gb1 = np.concatenate([np.asarray(g1, np.float32),
                          np.asarray(beta1, np.float32)])[None, :]
    av2 = np.stack([np.asarray(as2, np.float32), np.asarray(ad2, np.float32)], axis=1)
    in3 = [{"agg": agg1[c], "parts": parts1, "gb": gb1,
            "Wn": np.asarray(W2, np.float32), "avec": av2} for c in range(NCORE)]
    r3 = _run(nc3, in3, "L3")
    h2 = np.concatenate([r3[c]["out"] for c in range(NCORE)], axis=0)

    # ---- E2
    ine2 = [{"table": np.roll(h2, -PC * c, axis=0).copy(),
             "src_idx": src_arrs[c], "dst_loc": dst_arrs[c]} for c in range(NCORE)]
    re2 = _run(nce, ine2, "E2")
    agg2 = [re2[c]["agg"] for c in range(NCORE)]
    parts2 = np.stack([re2[c]["stats"][0] for c in range(NCORE)], axis=0)

    # ---- L5
    gb2 = np.concatenate([np.asarray(g2, np.float32),
                          np.asarray(beta2, np.float32)])[None, :]
    in5 = [{"agg": agg2[c], "parts": parts2, "gb": gb2,
            "Wn": np.asarray(Wc, np.float32),
            "bc": np.asarray(bc, np.float32)[None, :]} for c in range(NCORE)]
    r5 = _run(nc5, in5, "L5")
    logits = np.concatenate([r5[c]["out"] for c in range(NCORE)], axis=0)
    return logits[:N]


# revision 15
# speedup vs baseline: 1.2554x; 1.0078x over previous
"""GAT 2-layer + BN + classifier on 8 TRN2 NeuronCores (Bass/Tile).

Strategy: dst-block sharding with per-core table rotation so the SPMD
instruction stream is identical across cores. 5 launches:
  L1   node: h1_aug = x @ [W1 | W1@as1 | W1@ad1], table rows [h|as|ad|1|0]
  E(1) edge: gather h1_aug[src] per edge subtile, segment softmax via
       selection-matrix matmul in PSUM, partial BN stats
  L3   node: BN1 apply + relu + @W2_aug -> h2_aug table shard
  E(2) edge: same NEFF as E(1) on h2_aug
  L5   node: BN2 apply + relu + @Wc + bc -> logits shard
Host work is index-only: edge sort/shard, table assembly/rotation.
"""
import sys
sys.path.insert(0, '/opt/trn_rl_repo')
sys.path.insert(0, '/root/.axon_site')
import numpy as np

import concourse.bass as bass
import concourse.bacc as bacc
import concourse.tile as tile
from concourse import mybir
from concourse.masks import make_identity

F32 = mybir.dt.float32
I32 = mybir.dt.int32

N = 100000
NCORE = 8
BLK = 128
NPAD = 100352            # 784 blocks of 128
PC = NPAD // NCORE       # 12544 nodes/core = 98 blocks
NBLK = PC // BLK         # 98
TCOLS = 132              # table row: [h(128) | as | ad | one | pad]
HID = 128
NCLS = 40
NEG = 0.2
EPS = 1e-5

_EXEC_NS = []            # per-launch exec times when profiling enabled
PROFILE = False


RUN_HOOK = None          # test harness may set this to a profiling runner


def _run(nc, in_maps, label):
    if RUN_HOOK is not None:
        return RUN_HOOK(nc, in_maps, label)
    from concourse import bass2jax
    return bass2jax.run_bass_via_pjrt(nc, in_maps, n_cores=NCORE)


# ---------------------------------------------------------------- L1 node
def build_l1():
    nc = bacc.Bacc("TRN2", target_bir_lowering=False, debug=False, num_devices=NCORE)
    xT = nc.dram_tensor("xT", [128, NPAD], F32, kind="ExternalInput")
    W1 = nc.dram_tensor("W1", [128, HID], F32, kind="ExternalInput")
    avec = nc.dram_tensor("avec", [128, 2], F32, kind="ExternalInput")  # [as|ad] cols
    out = nc.dram_tensor("out", [PC, TCOLS], F32, kind="ExternalOutput")

    with tile.TileContext(nc) as tc:
        with (
            tc.tile_pool(name="c", bufs=1) as cp,
            tc.tile_pool(name="x", bufs=3) as xp,
            tc.tile_pool(name="o", bufs=3) as op,
            tc.tile_pool(name="ps", bufs=2, space="PSUM") as pp,
            tc.tile_pool(name="ps1", bufs=1, space="PSUM") as pp1,
        ):
            ident = cp.tile([128, 128], F32)
            make_identity(nc, ident[:])
            w_sb = cp.tile([128, HID], F32)
            nc.sync.dma_start(w_sb[:], W1[:])
            av_sb = cp.tile([128, 2], F32)
            nc.sync.dma_start(av_sb[:], avec[:])
            # W1T for v = W1 @ a
            wT_ps = pp1.tile([128, 128], F32, tag="tmp")
            nc.tensor.transpose(out=wT_ps[:], in_=w_sb[:], identity=ident[:])
            wT_sb = cp.tile([128, 128], F32)
            nc.vector.tensor_copy(out=wT_sb[:], in_=wT_ps[:])
            v_ps = pp1.tile([128, 2], F32, tag="tmp")
            nc.tensor.matmul(out=v_ps[:], lhsT=wT_sb[:], rhs=av_sb[:], start=True, stop=True)
            waug = cp.tile([128, HID + 2], F32)
            nc.vector.tensor_copy(out=waug[:, 0:HID], in_=w_sb[:])
            nc.vector.tensor_copy(out=waug[:, HID:HID + 2], in_=v_ps[:])
            GB = 4
            for g0 in range(0, NBLK, GB):
                nb = min(GB, NBLK - g0)
                xs = xp.tile([128, GB * 128], F32, tag="x", name=f"xs{g0}")
                nc.sync.dma_start(xs[:, 0:nb * 128],
                                  xT[:, g0 * 128:(g0 + nb) * 128])
                for i in range(nb):
                    t = g0 + i
                    h_ps = pp.tile([128, HID + 2], F32, tag="h", name=f"h{t}")
                    nc.tensor.matmul(out=h_ps[:], lhsT=xs[:, i * 128:(i + 1) * 128],
                                     rhs=waug[:], start=True, stop=True)
                    ot = op.tile([128, TCOLS], F32, tag="o", name=f"o{t}")
                    nc.vector.tensor_copy(out=ot[:, 0:HID + 2], in_=h_ps[:])
                    nc.vector.memset(ot[:, HID + 2:HID + 3], 1.0)
                    nc.vector.memset(ot[:, HID + 3:TCOLS], 0.0)
                    nc.scalar.dma_start(out[t * 128:(t + 1) * 128, :], ot[:])
    nc.compile()
    return nc


# ---------------------------------------------------------------- edge kernel
def build_edge(t_counts):
    """t_counts: list of NBLK subtile counts (shared across cores)."""
    nsub = int(sum(t_counts))
    nc = bacc.Bacc("TRN2", target_bir_lowering=False, debug=False, num_devices=NCORE)
    table = nc.dram_tensor("table", [NPAD, TCOLS], F32, kind="ExternalInput")
    src_idx = nc.dram_tensor("src_idx", [128, nsub], I32, kind="ExternalInput")
    dst_loc = nc.dram_tensor("dst_loc", [128, nsub], F32, kind="ExternalInput")
    agg = nc.dram_tensor("agg", [PC, HID], F32, kind="ExternalOutput")
    stats = nc.dram_tensor("stats", [1, 256], F32, kind="ExternalOutput")

    TMAX = max(t_counts)
    with tile.TileContext(nc) as tc:
        with (
            tc.tile_pool(name="c", bufs=1) as cp,
            tc.tile_pool(name="g", bufs=24) as gp,
            tc.tile_pool(name="s0", bufs=24) as s0p,
            tc.tile_pool(name="sw", bufs=3) as swp,
            tc.tile_pool(name="w", bufs=2) as wp,
            tc.tile_pool(name="ob", bufs=3) as obp,
            tc.tile_pool(name="own", bufs=2) as ownp,
            tc.tile_pool(name="pblk", bufs=2, space="PSUM") as pblk,
            tc.tile_pool(name="pal", bufs=2, space="PSUM") as pal,
            tc.tile_pool(name="ptr", bufs=2, space="PSUM") as ptr,
            tc.tile_pool(name="pst", bufs=1, space="PSUM") as pst,
        ):
            ident = cp.tile([128, 128], F32)
            make_identity(nc, ident[:])
            iota_i = cp.tile([128, 128], I32)
            nc.gpsimd.iota(iota_i[:], pattern=[[1, 128]], base=0, channel_multiplier=0)
            iota_f = cp.tile([128, 128], F32)
            nc.vector.tensor_copy(out=iota_f[:], in_=iota_i[:])
            ones_col = cp.tile([128, 1], F32)
            nc.vector.memset(ones_col[:], 1.0)
            idx_sb = cp.tile([128, nsub], I32)
            nc.sync.dma_start(idx_sb[:], src_idx[:])
            dl_sb = cp.tile([128, nsub], F32)
            nc.sync.dma_start(dl_sb[:], dst_loc[:])

            ps_sum = pst.tile([1, 128], F32, tag="sum")
            ps_sq = pst.tile([1, 128], F32, tag="sq")

            # zero-init gather buffers (stale-data guard)
            for i in range(24):
                gi = gp.tile([128, TCOLS], F32, tag="g", name=f"ginit{i}")
                nc.vector.memset(gi[:], 0.0)

            q0 = 0
            for t in range(NBLK):
                T = t_counts[t]
                # own rows: alpha_d of the block's nodes (cols 128..132)
                ownt = ownp.tile([128, 4], F32, tag="own")
                nc.sync.dma_start(ownt[:], table[t * 128:(t + 1) * 128, HID:HID + 4])
                ps_a = pal.tile([128, TMAX], F32, tag="al")
                s0_list = []
                g_list = []
                for s in range(T):
                    q = q0 + s
                    g = gp.tile([128, TCOLS], F32, tag="g")
                    nc.gpsimd.indirect_dma_start(
                        out=g[:], out_offset=None, in_=table[:],
                        in_offset=bass.IndirectOffsetOnAxis(ap=idx_sb[:, q:q + 1], axis=0))
                    g_list.append(g)
                    s0 = s0p.tile([128, 128], F32, tag="s0")
                    nc.vector.tensor_scalar(
                        out=s0[:], in0=iota_f[:], scalar1=dl_sb[:, q:q + 1],
                        scalar2=None, op0=mybir.AluOpType.is_equal)
                    s0_list.append(s0)
                    s0t_ps = ptr.tile([128, 128], F32, tag="tr")
                    nc.tensor.transpose(out=s0t_ps[:], in_=s0[:], identity=ident[:])
                    s0t = swp.tile([128, 128], F32, tag="s0t")
                    nc.vector.tensor_copy(out=s0t[:], in_=s0t_ps[:])
                    # alpha_d per edge -> ps_a[:, s]
                    nc.tensor.matmul(out=ps_a[:, s:s + 1], lhsT=s0t[:],
                                     rhs=ownt[:, 1:2], start=True, stop=True)
                # w = exp(lrelu(alpha_s + alpha_d)) for the whole block
                w_blk = wp.tile([128, TMAX], F32, tag="w")
                ps_b = pblk.tile([128, HID + 3], F32, tag="blk")
                for s in range(T):
                    g = g_list[s]
                    # alpha = alpha_s (g col 128) + alpha_d (ps_a col s)
                    nc.vector.tensor_tensor(
                        out=w_blk[:, s:s + 1], in0=g[:, HID:HID + 1],
                        in1=ps_a[:, s:s + 1], op=mybir.AluOpType.add)
                wb2 = wp.tile([128, TMAX], F32, tag="w2")
                nc.vector.tensor_scalar(out=wb2[:, 0:T], in0=w_blk[:, 0:T],
                                        scalar1=NEG, scalar2=None,
                                        op0=mybir.AluOpType.mult)
                nc.vector.tensor_tensor(out=w_blk[:, 0:T], in0=w_blk[:, 0:T],
                                        in1=wb2[:, 0:T], op=mybir.AluOpType.max)
                nc.scalar.activation(out=w_blk[:, 0:T], in_=w_blk[:, 0:T],
                                     func=mybir.ActivationFunctionType.Exp)
                for s in range(T):
                    sw = swp.tile([128, 128], F32, tag="sw")
                    nc.vector.tensor_scalar(
                        out=sw[:], in0=s0_list[s][:], scalar1=w_blk[:, s:s + 1],
                        scalar2=None, op0=mybir.AluOpType.mult)
                    nc.tensor.matmul(out=ps_b[:], lhsT=sw[:], rhs=g_list[s][:, 0:HID + 3],
                                     start=(s == 0), stop=(s == T - 1))
                # normalize: num = ps_b[:, 0:128], den = ps_b[:, 130]
                den = wp.tile([128, 1], F32, tag="den")
                nc.vector.tensor_scalar(out=den[:], in0=ps_b[:, HID + 2:HID + 3],
                                        scalar1=0.0, scalar2=None,
                                        op0=mybir.AluOpType.is_equal)
                nc.vector.tensor_tensor(out=den[:], in0=den[:],
                                        in1=ps_b[:, HID + 2:HID + 3],
                                        op=mybir.AluOpType.add)
                rec = wp.tile([128, 1], F32, tag="rec")
                nc.vector.reciprocal(out=rec[:], in_=den[:])
                ob = obp.tile([128, HID], F32, tag="ob")
                nc.vector.tensor_scalar(out=ob[:], in0=ps_b[:, 0:HID], scalar1=rec[:],
                                        scalar2=None, op0=mybir.AluOpType.mult)
                nc.sync.dma_start(agg[t * 128:(t + 1) * 128, :], ob[:])
                sq = obp.tile([128, HID], F32, tag="sq")
                nc.scalar.activation(out=sq[:], in_=ob[:],
                                     func=mybir.ActivationFunctionType.Square)
                nc.tensor.matmul(out=ps_sum[:], lhsT=ones_col[:], rhs=ob[:],
                                 start=(t == 0), stop=(t == NBLK - 1))
                nc.tensor.matmul(out=ps_sq[:], lhsT=ones_col[:], rhs=sq[:],
                                 start=(t == 0), stop=(t == NBLK - 1))
                q0 += T
            st_sb = cp.tile([1, 256], F32)
            nc.vector.tensor_copy(out=st_sb[:, 0:128], in_=ps_sum[:])
            nc.vector.tensor_copy(out=st_sb[:, 128:256], in_=ps_sq[:])
            nc.sync.dma_start(stats[:], st_sb[:])
    nc.compile()
    return nc


# ---------------------------------------------------------------- node tail
def build_node2(classifier):
    """BN apply + relu (+ next-layer table build, or classifier)."""
    nc = bacc.Bacc("TRN2", target_bir_lowering=False, debug=False, num_devices=NCORE)
    agg = nc.dram_tensor("agg", [PC, HID], F32, kind="ExternalInput")
    parts = nc.dram_tensor("parts", [8, 256], F32, kind="ExternalInput")
    gb = nc.dram_tensor("gb", [1, 256], F32, kind="ExternalInput")  # [gamma|beta]
    if classifier:
        Wn = nc.dram_tensor("Wn", [128, NCLS], F32, kind="ExternalInput")
        bc = nc.dram_tensor("bc", [1, NCLS], F32, kind="ExternalInput")
        out = nc.dram_tensor("out", [PC, NCLS], F32, kind="ExternalOutput")
    else:
        Wn = nc.dram_tensor("Wn", [128, HID], F32, kind="ExternalInput")
        avec = nc.dram_tensor("avec", [128, 2], F32, kind="ExternalInput")
        out = nc.dram_tensor("out", [PC, TCOLS], F32, kind="ExternalOutput")

    with tile.TileContext(nc) as tc:
        with (
            tc.tile_pool(name="c", bufs=1) as cp,
            tc.tile_pool(name="x", bufs=3) as xp,
            tc.tile_pool(name="o", bufs=3) as op,
            tc.tile_pool(name="ps", bufs=2, space="PSUM") as pp,
            tc.tile_pool(name="ps1", bufs=2, space="PSUM") as pp1,
        ):
            ident = cp.tile([128, 128], F32)
            make_identity(nc, ident[:])
            parts_sb = cp.tile([8, 256], F32)
            nc.sync.dma_start(parts_sb[:], parts[:])
            ones8 = cp.tile([8, 1], F32)
            nc.vector.memset(ones8[:], 1.0)
            st_ps = pp1.tile([1, 256], F32, tag="tmp")
            nc.tensor.matmul(out=st_ps[:], lhsT=ones8[:], rhs=parts_sb[:], start=True, stop=True)
            stat = cp.tile([1, 256], F32)
            nc.vector.tensor_scalar(out=stat[:], in0=st_ps[:], scalar1=1.0 / N,
                                    scalar2=None, op0=mybir.AluOpType.mult)
            mean = stat[:, 0:128]
            msq = stat[:, 128:256]
            var = cp.tile([1, 128], F32)
            nc.vector.tensor_tensor(out=var[:], in0=mean, in1=mean, op=mybir.AluOpType.mult)
            nc.vector.tensor_tensor(out=var[:], in0=msq, in1=var[:], op=mybir.AluOpType.subtract)
            nc.vector.tensor_scalar(out=var[:], in0=var[:], scalar1=EPS,
                                    scalar2=None, op0=mybir.AluOpType.add)
            std = cp.tile([1, 128], F32)
            nc.scalar.activation(out=std[:], in_=var[:],
                                 func=mybir.ActivationFunctionType.Sqrt)
            istd = cp.tile([1, 128], F32)
            nc.vector.reciprocal(out=istd[:], in_=std[:])
            gb_sb = cp.tile([1, 256], F32)
            nc.sync.dma_start(gb_sb[:], gb[:])
            gam = cp.tile([1, 128], F32)
            nc.vector.tensor_tensor(out=gam[:], in0=gb_sb[:, 0:128], in1=istd[:],
                                    op=mybir.AluOpType.mult)
            bet = cp.tile([1, 128], F32)
            nc.vector.tensor_tensor(out=bet[:], in0=mean, in1=gam[:], op=mybir.AluOpType.mult)
            nc.vector.tensor_tensor(out=bet[:], in0=gb_sb[:, 128:256], in1=bet[:],
                                    op=mybir.AluOpType.subtract)
            # broadcast gamma', beta' to [128, 128] via K=1 matmul
            one1 = cp.tile([1, 128], F32)
            nc.vector.memset(one1[:], 1.0)
            gbc_ps = pp1.tile([128, 128], F32, tag="tmp")
            nc.tensor.matmul(out=gbc_ps[:], lhsT=one1[:], rhs=gam[:], start=True, stop=True)
            gbc = cp.tile([128, 128], F32)
            nc.vector.tensor_copy(out=gbc[:], in_=gbc_ps[:])
            bbc_ps = pp1.tile([128, 128], F32, tag="tmp")
            nc.tensor.matmul(out=bbc_ps[:], lhsT=one1[:], rhs=bet[:], start=True, stop=True)
            bbc = cp.tile([128, 128], F32)
            nc.vector.tensor_copy(out=bbc[:], in_=bbc_ps[:])

            if classifier:
                wn_sb = cp.tile([128, NCLS], F32)
                nc.sync.dma_start(wn_sb[:], Wn[:])
                bc_sb = cp.tile([1, NCLS], F32)
                nc.sync.dma_start(bc_sb[:], bc[:])
                bcb_ps = pp1.tile([128, NCLS], F32, tag="tmp")
                nc.tensor.matmul(out=bcb_ps[:], lhsT=one1[:], rhs=bc_sb[:], start=True, stop=True)
                bcb = cp.tile([128, NCLS], F32)
                nc.vector.tensor_copy(out=bcb[:], in_=bcb_ps[:])
                rhs_w = wn_sb
                ncols = NCLS
            else:
                wn_sb = cp.tile([128, HID], F32)
                nc.sync.dma_start(wn_sb[:], Wn[:])
                av_sb = cp.tile([128, 2], F32)
                nc.sync.dma_start(av_sb[:], avec[:])
                wT_ps = pp1.tile([128, 128], F32, tag="tmp")
                nc.tensor.transpose(out=wT_ps[:], in_=wn_sb[:], identity=ident[:])
                wT_sb = cp.tile([128, 128], F32)
                nc.vector.tensor_copy(out=wT_sb[:], in_=wT_ps[:])
                v_ps = pp1.tile([128, 2], F32, tag="tmp")
                nc.tensor.matmul(out=v_ps[:], lhsT=wT_sb[:], rhs=av_sb[:], start=True, stop=True)
                waug = cp.tile([128, HID + 2], F32)
                nc.vector.tensor_copy(out=waug[:, 0:HID], in_=wn_sb[:])
                nc.vector.tensor_copy(out=waug[:, HID:HID + 2], in_=v_ps[:])
                rhs_w = waug
                ncols = HID + 2

            for t in range(NBLK):
                at = xp.tile([128, HID], F32, tag="a")
                nc.sync.dma_start(at[:], agg[t * 128:(t + 1) * 128, :])
                x2 = xp.tile([128, HID], F32, tag="x2")
                nc.vector.tensor_tensor(out=x2[:], in0=at[:], in1=gbc[:], op=mybir.AluOpType.mult)
                nc.vector.tensor_tensor(out=x2[:], in0=x2[:], in1=bbc[:], op=mybir.AluOpType.add)
                nc.scalar.activation(out=x2[:], in_=x2[:],
                                     func=mybir.ActivationFunctionType.Relu)
                xT_ps = pp.tile([128, 128], F32, tag="xt")
                nc.tensor.transpose(out=xT_ps[:], in_=x2[:], identity=ident[:])
                xT_sb = xp.tile([128, 128], F32, tag="xts")
                nc.vector.tensor_copy(out=xT_sb[:], in_=xT_ps[:])
                h_ps = pp.tile([128, ncols], F32, tag="h")
                nc.tensor.matmul(out=h_ps[:], lhsT=xT_sb[:], rhs=rhs_w[:], start=True, stop=True)
                if classifier:
                    ot = op.tile([128, NCLS], F32, tag="o")
                    nc.vector.tensor_tensor(out=ot[:], in0=h_ps[:], in1=bcb[:],
                                            op=mybir.AluOpType.add)
                    nc.scalar.dma_start(out[t * 128:(t + 1) * 128, :], ot[:])
                else:
                    ot = op.tile([128, TCOLS], F32, tag="o")
                    nc.vector.tensor_copy(out=ot[:, 0:HID + 2], in_=h_ps[:])
                    nc.vector.memset(ot[:, HID + 2:HID + 3], 1.0)
                    nc.vector.memset(ot[:, HID + 3:TCOLS], 0.0)
                    nc.scalar.dma_start(out[t * 128:(t + 1) * 128, :], ot[:])
    nc.compile()
    return nc


# ---------------------------------------------------------------- host glue
def _edge_arrays(src, dst):
    """Build per-core src_idx/dst_local arrays + shared t_counts."""
    order = np.argsort(dst, kind="stable")
    srcs = src[order]
    dsts = dst[order]
    blk = (dsts // BLK).astype(np.int64)
    counts = np.bincount(blk, minlength=NPAD // BLK)
    starts = np.concatenate([[0], np.cumsum(counts)])
    # shared subtile counts per slot t: max over cores
    cnt_mat = counts.reshape(NCORE, NBLK)
    t_counts = np.maximum(np.ceil(cnt_mat / BLK).astype(np.int64).max(axis=0), 1)
    nsub = int(t_counts.sum())
    offs = np.concatenate([[0], np.cumsum(t_counts)])
    src_arrs, dst_arrs = [], []
    for c in range(NCORE):
        si = np.zeros((128, nsub), np.int32)
        dl = np.full((128, nsub), 200.0, np.float32)
        for t in range(NBLK):
            b = c * NBLK + t
            s0, e0 = starts[b], starts[b + 1]
            cnt = e0 - s0
            if cnt == 0:
                continue
            k = np.arange(cnt)
            p = k % 128
            q = offs[t] + k // 128
            rolled = (srcs[s0:e0] - PC * c) % NPAD
            si[p, q] = rolled.astype(np.int32)
            dl[p, q] = (dsts[s0:e0] - b * BLK).astype(np.float32)
        src_arrs.append(si)
        dst_arrs.append(dl)
    return t_counts, src_arrs, dst_arrs


_CACHE = {}


def kernel(x, edge_index, W1, as1, ad1, b1, g1, beta1,
           W2, as2, ad2, b2, g2, beta2, Wc, bc):
    x = np.asarray(x, np.float32)
    ei = np.asarray(edge_index)
    src = np.concatenate([ei[0], np.arange(N, dtype=ei.dtype)]).astype(np.int64)
    dst = np.concatenate([ei[1], np.arange(N, dtype=ei.dtype)]).astype(np.int64)

    t_counts, src_arrs, dst_arrs = _edge_arrays(src, dst)

    key = tuple(t_counts.tolist())
    if key not in _CACHE:
        _CACHE[key] = (build_l1(), build_edge(t_counts),
                       build_node2(False), build_node2(True))
    nc1, nce, nc3, nc5 = _CACHE[key]

    # ---- L1
    xT = np.zeros((128, NPAD), np.float32)
    xT[:, :N] = np.asarray(x, np.float32).T
    av = np.stack([np.asarray(as1, np.float32), np.asarray(ad1, np.float32)], axis=1)
    in1 = [{"xT": np.roll(xT, -PC * c, axis=1).copy(),
            "W1": np.asarray(W1, np.float32), "avec": av} for c in range(NCORE)]
    r1 = _run(nc1, in1, "L1")
    h1 = np.concatenate([r1[c]["out"] for c in range(NCORE)], axis=0)  # [NPAD, 132]

    # ---- E1
    ine = [{"table": np.roll(h1, -PC * c, axis=0).copy(),
            "src_idx": src_arrs[c], "dst_loc": dst_arrs[c]} for c in range(NCORE)]
    re1 = _run(nce, ine, "E1")
    agg1 = [re1[c]["agg"] for c in range(NCORE)]
    parts1 = np.stack([re1[c]["stats"][0] for c in range(NCORE)], axis=0)  # [8, 256]

    # ---- L3
    gb1 = np.concatenate([np.asarray(g1, np.float32),
                          np.asarray(beta1, np.float32)])[None, :]
    av2 = np.stack([np.asarray(as2, np.float32), np.asarray(ad2, np.float32)], axis=1)
    in3 = [{"agg": agg1[c], "parts": parts1, "gb": gb1,
            "Wn": np.asarray(W2, np.float32), "avec": av2} for c in range(NCORE)]
    r3 = _run(nc3, in3, "L3")
    h2 = np.concatenate([r3[c]["out"] for c in range(NCORE)], axis=0)

    # ---- E2
    ine2 = [{"table": np.roll(h2, -PC * c, axis=0).copy(),
             "src_idx": src_arrs[c], "dst_loc": dst_arrs[c]} for c in range(NCORE)]
    re2 = _run(nce, ine2, "E2")
    agg2 = [re2[c]["agg"] for c in range(NCORE)]
    parts2 = np.stack([re2[c]["stats"][0] for c in range(NCORE)], axis=0)

    # ---- L5
    gb2 = np.concatenate([np.asarray(g2, np.float32),
                          np.asarray(beta2, np.float32)])[None, :]
    in5 = [{"agg": agg2[c], "parts": parts2, "gb": gb2,
            "Wn": np.asarray(Wc, np.float32),
            "bc": np.asarray(bc, np.float32)[None, :]} for c in range(NCORE)]
    r5 = _run(nc5, in5, "L5")
    logits = np.concatenate([r5[c]["out"] for c in range(NCORE)], axis=0)
    return logits[:N]
